# revision 58
# baseline (speedup 1.0000x reference)
"""Lorentz MLA attention kernel for Trainium2, sharded over 8 NeuronCores.

Sharding: tensor-parallel over the 16 attention heads (2 heads per core);
the kv_lora latent projection (wkv_a + RMS norm) is sequence-sharded and
AllGathered. The output projection wo is row-parallel: each core produces a
partial (2048, 2047) output in bf16; the host sums the 8 partials in f32 and
applies the final Lorentz lift.

Device-side layout notes:
- Everything flows transposed ([feature, s]) so all matmuls contract on the
  partition axis without transposing x on device (host passes x^T).
- The 193-dim Lorentz q/k contraction is chunked [128 nope] + [64 rope + 1
  time]; the time rows sit at partition 64 of the 65-row "B" tiles.
- Rope dims are permuted even-pairs-first on the host so rotary is aligned
  [32, n] ops; weight columns are permuted to match.
- All matmul operands are bf16 (f32r at <256 moving columns runs at 1/4 PE
  rate); PSUM accumulation stays f32.
- Scalar engine uses ONLY the natural_log_exp activation table: every sqrt
  is computed as exp(0.5*ln(1+x)) so no ACT table reloads ever happen.
- Softmax max-pass skipped (scores <= 0 on the hyperboloid) and the softmax
  denominator cancels inside the Lorentz centroid normalization. V' carries
  its time coordinate in column 127 (wo rows are permuted on host to match).
- AV runs Vp-stationary: one N=512 matmul per j-tile accumulates aveT [d, q]
  in PSUM, so the centroid epilogue and the wo projection need no transposes.
  Scores for two j-tiles share one 2-bank PSUM tile; each exp call covers
  1024 columns, halving the scalar engine's per-call pipeline-fill cost.
- q/k time rows are batched: one-hot column-selector matmuls accumulate all
  (head, chunk) |.|^2 sums into an [8, 512] PSUM tile, so one ln/exp pair
  replaces sixteen 1-lane activation calls; squares run on the DVE (dual
  SBUF reads at 2x fp16 rate) instead of the scalar engine.
- The attention pair-loop is software-pipelined, and the previous group's wo
  matmuls are sprinkled between the score and (exp-gated) AV matmuls: the PE
  queue always holds independent work ahead of a semaphore wait, so it never
  micro-idles (micro-gaps hold the HAM clock gate at the 1.2 GHz K=4/8
  state; dense streams keep the 13/16 GPIO-limited 1.95 GHz).
- V' tiles are produced with PE transposes, drained on the scalar engine.
"""

import os
import sys
import types

import numpy as np
import ml_dtypes


def _ensure_axon_hooks():
    """Recreate the missing antenv.axon_hooks module so NTFF tracing works."""
    if "antenv.axon_hooks" in sys.modules:
        return
    try:
        import antenv
        from trn_agent_boot.trn_boot import _ntff_profile_via_ctypes

        hook = _ntff_profile_via_ctypes("/opt/axon/libaxon_pjrt.so")
        mod = types.ModuleType("antenv.axon_hooks")
        mod.get_axon_ntff_profile_hook = lambda: hook
        mod.set_axon_ntff_profile_hook = lambda h: None
        sys.modules["antenv.axon_hooks"] = mod
        antenv.axon_hooks = mod
    except Exception:
        pass


_ensure_axon_hooks()

import concourse.bacc as bacc
import concourse.bass as bass
import concourse.tile as tile
from concourse import mybir
import concourse.bass_utils as bass_utils
from concourse.bass_utils import run_bass_kernel_spmd
from concourse.masks import make_identity, make_upper_triangular

# zero-egress container: make the S3 artifact upload in the profile path a no-op
bass_utils.upload_artifacts = lambda tmpdir: tmpdir

F32 = mybir.dt.float32
BF16 = mybir.dt.float16  # 16-bit compute dtype (fp16: 10 mantissa bits)
FP8 = mybir.dt.float8e4  # e4m3, used for the latent gather payload
AF = mybir.ActivationFunctionType
AX = mybir.AxisListType
ALU = mybir.AluOpType

N_CORES = 8
P = 128
S = 2048          # sequence length
DIM = 2048        # model dim
NDC = DIM // P    # 16 contraction chunks over DIM
NQT = S // P      # 16 q/k tiles of 128
HPC = 2           # heads per core
NOPE = 128
RSP = 64          # rotary space dim
VSP = 127         # v space dim
KV_RANK = 512
EPS_RMS = 1e-6
QH = NOPE + RSP               # 192 q space rows per head
WQ_COLS = HPC * QH            # 384
WB_COLS = HPC * (NOPE + VSP)  # 510
WO_ROWS = HPC * P             # 256
OUT_COLS = DIM - 1            # 2047
NCH = 512                     # column chunk
NA = S // NCH                 # 4 chunks
SL = S // N_CORES             # 256
GR = KV_RANK + RSP + 1        # gathered rows: kvn + kpe + t_row


def _build_program(exp_scale: float, causal: bool):
    nc = bacc.Bacc("TRN2", target_bir_lowering=False, debug=False,
                   num_devices=N_CORES)

    xT_d = nc.dram_tensor("xT", [DIM, S], BF16, kind="ExternalInput")
    wq_d = nc.dram_tensor("wq", [DIM, WQ_COLS], BF16, kind="ExternalInput")
    wkva_d = nc.dram_tensor("wkva", [DIM, KV_RANK + RSP], BF16, kind="ExternalInput")
    wnormT_d = nc.dram_tensor("wnormT", [P, 4], F32, kind="ExternalInput")
    wkvb_d = nc.dram_tensor("wkvb", [KV_RANK + 1, WB_COLS], BF16, kind="ExternalInput")
    wo8_d = nc.dram_tensor("wo8", [P, 2, 2048], FP8, kind="ExternalInput")
    cosT_d = nc.dram_tensor("cosT", [RSP, S], BF16, kind="ExternalInput")
    sinT_d = nc.dram_tensor("sinT", [RSP, S], BF16, kind="ExternalInput")
    l2_d = nc.dram_tensor("l2c", [P, 1], F32, kind="ExternalInput")
    lcc_d = nc.dram_tensor("lcc", [P, 128], BF16, kind="ExternalInput")
    out_d = nc.dram_tensor("out", [S, OUT_COLS], BF16, kind="ExternalOutput")
    xsl_d = nc.dram_tensor("xsl", [DIM, SL], BF16, kind="ExternalInput")
    cossl_d = nc.dram_tensor("cossl", [RSP, SL], BF16, kind="ExternalInput")
    sinsl_d = nc.dram_tensor("sinsl", [RSP, SL], BF16, kind="ExternalInput")
    gin = nc.dram_tensor("gin", [GR, SL], BF16)
    gout = nc.dram_tensor("gout", [N_CORES, GR, SL], BF16, addr_space="Shared")

    with tile.TileContext(nc) as tc:
        if os.environ.get("LMLA_NO_TABLE_PRELOAD") != "1":
            # Preload the combined ln+exp ACT table once; without this the
            # auto-placement pass alternates exp->table0 / ln->table5 loads
            # (1.28us each) all through the attention loop.
            nc.scalar.add_instruction(mybir.InstLoadActFuncSet(
                name=nc.get_next_instruction_name(), act_func_set_id=6,
                ins=[], outs=[]))
        const = tc.alloc_tile_pool(name="const", bufs=1)
        identity = const.tile([P, P], BF16)
        make_identity(nc, identity)
        diagmask = const.tile([P, P], BF16)
        make_upper_triangular(nc, diagmask, val=1.0, diag=True)
        wnormT = const.tile([P, 4], F32)
        nc.sync.dma_start(out=wnormT[:], in_=wnormT_d[:])
        Lt = const.tile([P, 4, 2], BF16)  # [ones | wnorm^2] per latent chunk
        for c in range(4):
            nc.vector.memset(Lt[:, c, 0:1], 1.0)
            nc.vector.tensor_mul(Lt[:, c, 1:2], wnormT[:, c:c + 1], wnormT[:, c:c + 1])
        ones_col = const.tile([P, 1], BF16)
        nc.vector.memset(ones_col[:], 1.0)
        ones_row = const.tile([1, P], F32)
        nc.vector.memset(ones_row[:], 1.0)
        ones_row_bf = const.tile([1, P], BF16)
        nc.vector.memset(ones_row_bf[:], 1.0)
        eps_b = const.tile([P, 1], F32)
        nc.vector.memset(eps_b[:], EPS_RMS)
        ln16_b = const.tile([P, 1], F32)
        nc.vector.memset(ln16_b[:], 2.772588722239781)

        # Long-lived tiles.
        big = tc.alloc_tile_pool(name="big", bufs=1)
        qsA = [big.tile([P, S], BF16, name=f"qsA_{h}", tag=f"qsA_{h}") for h in range(HPC)]
        qsB = [big.tile([RSP + 1, S], BF16, name=f"qsB_{h}", tag=f"qsB_{h}") for h in range(HPC)]
        kv = [big.tile([P, S], BF16, name=f"kv_{c}", tag=f"kv_{c}") for c in range(4)]
        kpe = big.tile([RSP, S], BF16, name="kpe", tag="kpe")
        ksB = [big.tile([RSP + 1, S], BF16, name=f"ksB_{h}", tag=f"ksB_{h}") for h in range(HPC)]
        Vp = [big.tile([P, NQT, P], BF16, name=f"Vp_{h}", tag=f"Vp_{h}") for h in range(HPC)]
        t_row_bf = big.tile([1, S], BF16, name="t_row_bf", tag="t_row_bf")

        # ------------- Slice phase: kv latent on this core's s-slice ---------
        p_wKV = tc.alloc_tile_pool(name="p_wKV", bufs=1)
        p_sl = tc.alloc_tile_pool(name="p_sl", bufs=1)
        p_pssl = tc.alloc_tile_pool(name="p_pssl", bufs=2, space="PSUM")
        wKV = []
        for dc in range(NDC):
            w = p_wKV.tile([P, KV_RANK + RSP], BF16, name=f"wKV_{dc}", tag=f"wKV_{dc}")
            nc.sync.dma_start(out=w[:], in_=wkva_d[dc * P:(dc + 1) * P, :])
            wKV.append(w)
        xsl_t = p_sl.tile([P, NDC, SL], BF16, name="xsl_t", tag="xsl_t")
        for dc in range(NDC):
            nc.sync.dma_start(out=xsl_t[:, dc, :],
                              in_=xsl_d[dc * P:(dc + 1) * P, :])
        cossl = p_sl.tile([RSP, SL], BF16, name="cossl", tag="cossl")
        sinsl = p_sl.tile([RSP, SL], BF16, name="sinsl", tag="sinsl")
        nc.sync.dma_start(out=cossl[:], in_=cossl_d[:])
        nc.sync.dma_start(out=sinsl[:], in_=sinsl_d[:])

        # phase-A weights prefetch during the slice compute (pure loads, no
        # waits, so they issue immediately on sync)
        p_wA = tc.alloc_tile_pool(name="p_wA", bufs=1)
        p_qsc = tc.alloc_tile_pool(name="p_qsc", bufs=1)
        cosT = p_qsc.tile([RSP, S], BF16, name="cosT", tag="cosT")
        sinT = p_qsc.tile([RSP, S], BF16, name="sinT", tag="sinT")
        nc.sync.dma_start(out=cosT[:], in_=cosT_d[:])
        nc.sync.dma_start(out=sinT[:], in_=sinT_d[:])
        wQ = []
        for dc in range(NDC):
            w = p_wA.tile([P, WQ_COLS], BF16, name=f"wQ_{dc}", tag=f"wQ_{dc}")
            nc.sync.dma_start(out=w[:], in_=wq_d[dc * P:(dc + 1) * P, :])
            wQ.append(w)

        kvsl = [p_sl.tile([P, SL], F32, name=f"kvsl_{c}", tag=f"kvsl_{c}")
                for c in range(4)]
        kpesl = p_sl.tile([RSP, SL], F32, name="kpesl", tag="kpesl")
        for c in range(4):
            ps = p_pssl.tile([P, SL], F32, name="psl", tag="psl", bufs=2)
            for dc in range(NDC):
                nc.tensor.matmul(ps[:], wKV[dc][:, c * P:(c + 1) * P],
                                 xsl_t[:, dc, :], start=(dc == 0), stop=(dc == NDC - 1))
            nc.vector.tensor_copy(kvsl[c][:], ps[:])
        ps = p_pssl.tile([P, SL], F32, name="psl", tag="psl", bufs=2)
        for dc in range(NDC):
            nc.tensor.matmul(ps[:RSP, :], wKV[dc][:, KV_RANK:],
                             xsl_t[:, dc, :], start=(dc == 0), stop=(dc == NDC - 1))
        nc.vector.tensor_copy(kpesl[:], ps[:RSP, :])

        # RMS stats on the slice
        ps_s = p_pssl.tile([1, SL], F32, name="ps_s", tag="ps_s", bufs=1)
        ps_w = p_pssl.tile([1, SL], F32, name="ps_w", tag="ps_w", bufs=1)
        for c in range(4):
            ksq = p_sl.tile([P, SL], BF16, name="ksq", tag="ksq", bufs=2)
            nc.scalar.square(ksq[:], kvsl[c][:])
            nc.tensor.matmul(ps_s[:], Lt[:, c, 0:1], ksq[:], start=(c == 0), stop=(c == 3))
            nc.tensor.matmul(ps_w[:], Lt[:, c, 1:2], ksq[:], start=(c == 0), stop=(c == 3))
        # inv_rms = exp(-0.5 * ln(mean_sq + eps)) ; single ACT table (ln/exp)
        ln_s = p_sl.tile([1, SL], F32, name="ln_s", tag="ln_s")
        nc.scalar.activation(ln_s[:], ps_s[:], AF.Ln, bias=eps_b[0:1, :],
                             scale=1.0 / KV_RANK)
        inv_rms = p_sl.tile([1, SL], F32, name="inv_rms", tag="inv_rms")
        nc.scalar.activation(inv_rms[:], ln_s[:], AF.Exp, scale=-0.5)
        tmp_r = p_sl.tile([1, SL], F32, name="tmp_r", tag="tmp_r")
        nc.vector.tensor_copy(tmp_r[:], ps_w[:])
        nc.vector.tensor_mul(tmp_r[:], tmp_r[:], inv_rms[:])
        nc.vector.tensor_mul(tmp_r[:], tmp_r[:], inv_rms[:])
        t_ln = p_sl.tile([1, SL], F32, name="t_ln", tag="t_ln")
        nc.scalar.activation(t_ln[:], tmp_r[:], AF.Ln, bias=1.0)
        t_st = p_sl.tile([1, SL], BF16, name="t_st", tag="t_st")
        nc.scalar.activation(t_st[:], t_ln[:], AF.Exp, scale=0.5)

        # broadcast inv_rms via outer product; fused scale -> bf16 stage
        rb = p_pssl.tile([P, SL], F32, name="rb", tag="rb", bufs=1)
        nc.tensor.matmul(rb[:], ones_row[:], inv_rms[:], start=True, stop=True)
        kvn_st = [p_sl.tile([P, SL], BF16, name=f"kvn_st_{c}", tag=f"kvn_st_{c}")
                  for c in range(4)]
        for c in range(4):
            nc.vector.scalar_tensor_tensor(
                kvn_st[c][:], kvsl[c][:], wnormT[:, c:c + 1], rb[:],
                op0=ALU.mult, op1=ALU.mult)

        # rotary on the k_pe slice
        rtl = p_sl.tile([RSP, SL], F32, name="rtl", tag="rtl")
        kpe_st = p_sl.tile([RSP, SL], BF16, name="kpe_st", tag="kpe_st")
        x0 = kpesl[0:32, :]
        x1 = kpesl[32:64, :]
        nc.vector.tensor_mul(rtl[32:64, :], x0, sinsl[0:32, :])
        nc.vector.tensor_mul(rtl[0:32, :], x1, sinsl[32:64, :])
        nc.vector.tensor_mul(x0, x0, cossl[0:32, :])
        nc.vector.tensor_mul(x1, x1, cossl[32:64, :])
        nc.vector.tensor_sub(kpe_st[0:32, :], x0, rtl[0:32, :])
        nc.vector.tensor_add(kpe_st[32:64, :], x1, rtl[32:64, :])

        # ship slice, gather full (single bf16 payload). The gin writes go on
        # the vector queue (their producers): on the in-order sync queue they
        # would block the phase-A weight/x DMA issues behind the slice tail.
        for c in range(4):
            nc.gpsimd.dma_start(out=gin[c * P:(c + 1) * P, :], in_=kvn_st[c][:])
        nc.gpsimd.dma_start(out=gin[KV_RANK:KV_RANK + RSP, :], in_=kpe_st[:])
        nc.gpsimd.dma_start(out=gin[KV_RANK + RSP:, :], in_=t_st[:])
        nc.gpsimd.collective_compute(
            "AllGather", ALU.bypass,
            replica_groups=[list(range(N_CORES))],
            ins=[gin[:]], outs=[gout[:]])
        # keep p_sl/p_wKV alive through phase A: recycling their SBUF for the
        # xt tiles makes the PE wait on the slice tail + gin DMA reads
        p_pssl.release()

        # --- Phase A: q projection over the full sequence --------------------
        # weight col layout (host): [qnope0 | qnope1 | qrope0(ev,od) | qrope1]
        # Per n-chunk: matmuls, drains to bf16, rotary (vector). q/k/v time
        # rows are all computed in phase B from the bf16 SBUF copies (DVE
        # squares + batched [8, 512] ln/exp).
        p_xs = tc.alloc_tile_pool(name="p_xs", bufs=1)
        p_psA = tc.alloc_tile_pool(name="p_psA", bufs=3, space="PSUM")

        for n in range(NA):
            n0 = n * NCH
            xt = p_xs.tile([P, NDC, NCH], BF16, name="xt", tag="xt", bufs=2)
            src = xT_d[:, n0:n0 + NCH].rearrange("(dc p) s -> p dc s", p=P)
            for dc in range(NDC):
                nc.sync.dma_start(out=xt[:, dc, :], in_=src[:, dc, :])

            # rope chunk for both heads: rows [h0ev|h0od|h1ev|h1od]
            ps = p_psA.tile([P, NCH], F32, name="psa", tag="psa", bufs=3)
            for dc in range(NDC):
                nc.tensor.matmul(ps[:], wQ[dc][:, 2 * P:3 * P], xt[:, dc, :],
                                 start=(dc == 0), stop=(dc == NDC - 1))
            for h in range(HPC):
                nc.scalar.copy(qsB[h][0:RSP, n0:n0 + NCH], ps[h * RSP:(h + 1) * RSP, :])
            # rotary, in place on bf16 (2x DVE mode)
            rt = p_qsc.tile([RSP, NCH], BF16, name="rt", tag="rt", bufs=2)
            for h in range(HPC):
                gx0 = qsB[h][0:32, n0:n0 + NCH]
                gx1 = qsB[h][32:64, n0:n0 + NCH]
                nc.vector.tensor_mul(rt[32:64, :], gx0, sinT[0:32, n0:n0 + NCH])
                nc.vector.tensor_mul(rt[0:32, :], gx1, sinT[32:64, n0:n0 + NCH])
                nc.vector.tensor_mul(gx0, gx0, cosT[0:32, n0:n0 + NCH])
                nc.vector.tensor_mul(gx1, gx1, cosT[32:64, n0:n0 + NCH])
                nc.vector.tensor_sub(gx0, gx0, rt[0:32, :])
                nc.vector.tensor_add(gx1, gx1, rt[32:64, :])

            for h in range(HPC):
                ps = p_psA.tile([P, NCH], F32, name="psa", tag="psa", bufs=3)
                for dc in range(NDC):
                    nc.tensor.matmul(ps[:], wQ[dc][:, h * P:(h + 1) * P],
                                     xt[:, dc, :], start=(dc == 0), stop=(dc == NDC - 1))
                nc.vector.tensor_copy(qsA[h][:, n0:n0 + NCH], ps[:])
        p_psA.release()
        p_xs.release()
        p_qsc.release()
        p_wA.release()
        p_sl.release()
        p_wKV.release()

        # gather unpack. Emitted AFTER the phase-A pool releases and on the
        # (idle) gpsimd engine: the triggers wait on the collective, so on the
        # in-order sync engine they'd starve phase A's xt loads, and if they
        # precede the releases the release drain (queued behind them on
        # gpsimd) gates every post-release allocation.
        # kv unpack split by 512-col output chunk (k-slot pairs) so phase B's
        # first chunk matmuls fire as soon as their slice of the gather lands
        for nn in range(NA):
            for c in range(4):
                nc.gpsimd.dma_start(
                    out=kv[c][:, nn * NCH:(nn + 1) * NCH].rearrange(
                        "p (k s) -> p k s", k=2),
                    in_=gout[2 * nn:2 * nn + 2, c * P:(c + 1) * P, :].rearrange(
                        "k p s -> p k s"))
        # k_pe rows are shared by both heads: unpack straight into both ksB
        # tiles (and once more into kpe for the k-time squares)
        for h in range(HPC):
            nc.gpsimd.dma_start(
                out=ksB[h][0:RSP, :].rearrange("p (k s) -> p k s", k=N_CORES),
                in_=gout[:, KV_RANK:KV_RANK + RSP, :].rearrange("k p s -> p k s"))
        nc.gpsimd.dma_start(
            out=kpe[:].rearrange("p (k s) -> p k s", k=N_CORES),
            in_=gout[:, KV_RANK:KV_RANK + RSP, :].rearrange("k p s -> p k s"))
        nc.gpsimd.dma_start(
            out=t_row_bf[:].rearrange("p (k s) -> p k s", k=N_CORES),
            in_=gout[:, KV_RANK + RSP:, :].rearrange("k p s -> p k s"))

        # --- Phase B: kv_b projection + k/v time rows + V' assembly ----------
        big2 = tc.alloc_tile_pool(name="big2", bufs=1)
        p_wB = tc.alloc_tile_pool(name="p_wB", bufs=1)
        p_psB = tc.alloc_tile_pool(name="p_psB", bufs=3, space="PSUM")
        p_pkv = tc.alloc_tile_pool(name="p_pkv", bufs=2, space="PSUM")
        p_ptv = tc.alloc_tile_pool(name="p_ptv", bufs=2, space="PSUM")
        p_bsc = tc.alloc_tile_pool(name="p_bsc", bufs=1)
        wb_k = []
        for k in range(4):
            w = p_wB.tile([P, WB_COLS], BF16, name=f"wbk_{k}", tag=f"wbk_{k}")
            nc.sync.dma_start(out=w[:], in_=wkvb_d[k * P:(k + 1) * P, :])
            wb_k.append(w)
        wb_t = p_wB.tile([1, WB_COLS], BF16, name="wb_t", tag="wb_t")
        nc.sync.dma_start(out=wb_t[:], in_=wkvb_d[KV_RANK:KV_RANK + 1, :])

        ksA = [big2.tile([P, S], BF16, name=f"ksA_{h}", tag=f"ksA_{h}") for h in range(HPC)]
        vts = [big2.tile([P, S], BF16, name=f"vts_{h}", tag=f"vts_{h}") for h in range(HPC)]

        def kvb_mms(ps, col0, msize, n0):
            for k in range(4):
                nc.tensor.matmul(ps[:msize, :], wb_k[k][:, col0:col0 + msize],
                                 kv[k][:, n0:n0 + NCH], start=(k == 0), stop=False)
            nc.tensor.matmul(ps[:msize, :], wb_t[:, col0:col0 + msize],
                             t_row_bf[:, n0:n0 + NCH], start=False, stop=True)

        # batched time-row accumulators: rows r = h*4 + n of [8, NCH]; a
        # single ln/exp pair then covers all (h, n) at once. The one-hot
        # column selectors come from the host (lcc): zero columns write
        # zeros to the other rows, which is harmless under accumulation.
        lcc = p_wB.tile([P, 128], BF16, name="lcc", tag="lcc")
        nc.sync.dma_start(out=lcc[:], in_=lcc_d[:])
        qkall = p_pkv.tile([8, NCH], F32, name="qkall", tag="qkall", bufs=1)
        pkall = p_pkv.tile([8, NCH], F32, name="pkall", tag="pkall", bufs=1)

        for n in range(NA):
            n0 = n * NCH
            # DVE squares from the bf16 SBUF copies (dual SBUF reads, 2x rate)
            kpsq = p_bsc.tile([RSP, NCH], BF16, name="kpsq", tag="kpsq", bufs=2)
            nc.vector.tensor_mul(kpsq[:], kpe[:, n0:n0 + NCH],
                                 kpe[:, n0:n0 + NCH])
            qsq = p_bsc.tile([P, NCH], BF16, name="qsq", tag="qsq", bufs=2)
            for h in range(HPC):
                nc.vector.tensor_mul(qsq[h * RSP:(h + 1) * RSP, :],
                                     qsB[h][0:RSP, n0:n0 + NCH],
                                     qsB[h][0:RSP, n0:n0 + NCH])
            nc.tensor.matmul(qkall[:], lcc[:, 96 + 8 * n:96 + 8 * n + 8],
                             qsq[:], start=(n == 0), stop=False,
                             skip_group_check=True)
            for h in range(HPC):
                r = h * 4 + n
                qbsq = p_bsc.tile([P, NCH], BF16, name="qbsq", tag="qbsq", bufs=2)
                nc.vector.tensor_mul(qbsq[:], qsA[h][:, n0:n0 + NCH],
                                     qsA[h][:, n0:n0 + NCH])
                nc.tensor.matmul(qkall[:], lcc[:, 8 * r:8 * r + 8], qbsq[:],
                                 start=False, stop=(n == NA - 1 and h == HPC - 1),
                                 skip_group_check=True)
            for h in range(HPC):
                c0 = h * (NOPE + VSP)
                r = h * 4 + n
                # k_nope
                ps = p_psB.tile([P, NCH], F32, name="psb", tag="psb", bufs=3)
                kvb_mms(ps, c0, NOPE, n0)
                nc.vector.tensor_copy(ksA[h][:, n0:n0 + NCH], ps[:])
                bsq = p_bsc.tile([P, NCH], BF16, name="bsq", tag="bsq", bufs=2)
                nc.vector.tensor_mul(bsq[:], ksA[h][:, n0:n0 + NCH],
                                     ksA[h][:, n0:n0 + NCH])
                nc.tensor.matmul(pkall[:], lcc[:, 8 * r:8 * r + 8], bsq[:],
                                 start=(n == 0 and h == 0), stop=False,
                                 skip_group_check=True)
                if h == HPC - 1:
                    nc.tensor.matmul(pkall[:], lcc[0:RSP, 64 + 8 * n:64 + 8 * n + 8],
                                     kpsq[:],
                                     start=False, stop=(n == NA - 1),
                                     skip_group_check=True)
                # v (127 space rows; time goes in row 127 of vts)
                ps = p_psB.tile([P, NCH], F32, name="psb", tag="psb", bufs=3)
                kvb_mms(ps, c0 + NOPE, VSP, n0)
                nc.vector.tensor_copy(vts[h][0:VSP, n0:n0 + NCH], ps[:VSP, :])
                vsq = p_bsc.tile([VSP, NCH], BF16, name="vsq", tag="vsq", bufs=2)
                nc.vector.tensor_mul(vsq[:], vts[h][0:VSP, n0:n0 + NCH],
                                     vts[h][0:VSP, n0:n0 + NCH])
                pv = p_pkv.tile([1, NCH], F32, name="pv", tag="pv", bufs=1)
                nc.tensor.matmul(pv[:], ones_col[0:VSP, :], vsq[:],
                                 start=True, stop=True)
                vln = p_bsc.tile([1, NCH], F32, name="vln", tag="vln", bufs=2)
                nc.scalar.activation(vln[:], pv[:], AF.Ln, bias=1.0)
                # engines can't write a region based at partition 127; go via
                # a scratch row + SBUF->SBUF DMA
                vtr = p_bsc.tile([1, NCH], BF16, name="vtr", tag="vtr", bufs=2)
                nc.scalar.activation(vtr[:], vln[:], AF.Exp, scale=0.5)
                nc.sync.dma_start(out=vts[h][VSP:VSP + 1, n0:n0 + NCH],
                                  in_=vtr[:])
                # V' tiles for this chunk: PE transposes (DMA xbar transposes
                # get scheduled lazily, serialize against other DMAs, and
                # stall the attention loop's AV matmuls)
                for j in range(n * 4, n * 4 + 4):
                    tpv = p_ptv.tile([P, P], BF16, name="tpv", tag="tpv", bufs=2)
                    nc.tensor.transpose(tpv[:], vts[h][:, j * P:(j + 1) * P],
                                        identity[:])
                    nc.scalar.copy(Vp[h][:, j, :], tpv[:])
        # finalize the time rows: one ln/exp pair per quantity
        kt8 = p_bsc.tile([8, NCH], BF16, name="kt8", tag="kt8")
        kl8 = p_bsc.tile([8, NCH], F32, name="kl8", tag="kl8")
        nc.scalar.activation(kl8[:], pkall[:], AF.Ln, bias=1.0)
        nc.scalar.activation(kt8[:], kl8[:], AF.Exp, scale=0.5)
        qt8 = p_bsc.tile([8, NCH], BF16, name="qt8", tag="qt8")
        ql8 = p_bsc.tile([8, NCH], F32, name="ql8", tag="ql8")
        nc.scalar.activation(ql8[:], qkall[:], AF.Ln, bias=1.0)
        nc.scalar.activation(qt8[:], ql8[:], AF.Exp, scale=0.5)
        qt8n = p_bsc.tile([8, NCH], BF16, name="qt8n", tag="qt8n")
        nc.vector.tensor_scalar_mul(qt8n[:], qt8[:], -1.0)
        for h in range(HPC):
            for n in range(NA):
                r = h * 4 + n
                n0 = n * NCH
                nc.gpsimd.dma_start(out=ksB[h][RSP:RSP + 1, n0:n0 + NCH],
                                    in_=kt8[r:r + 1, :])
                nc.gpsimd.dma_start(out=qsB[h][RSP:RSP + 1, n0:n0 + NCH],
                                    in_=qt8n[r:r + 1, :])
        p_bsc.release()
        p_ptv.release()
        p_pkv.release()
        p_psB.release()
        p_wB.release()

        # ---------------- Phase C: attention ---------------------------------
        # scoresT layout [k, q]. AV runs Vp-stationary: one N=512 matmul per
        # j-tile accumulating aveT [d, q] in PSUM, so the epilogue and the wo
        # projection need no transposes at all. Scores for two j-tiles land in
        # one 2-bank PSUM tile so each exp call covers 1024 columns (the
        # scalar engine's per-call pipeline fill is ~290ns). The pair-loop is
        # software-pipelined two deep so the PE never waits on the exp.
        GQ = NCH // P
        NG = S // NCH
        p_ex = tc.alloc_tile_pool(name="p_ex", bufs=4)
        p_cw = tc.alloc_tile_pool(name="p_cw", bufs=2)
        p_wO = tc.alloc_tile_pool(name="p_wO", bufs=1)
        p_osb = tc.alloc_tile_pool(name="p_osb", bufs=4)
        p_ave = tc.alloc_tile_pool(name="p_ave", bufs=1, space="PSUM")
        p_scp = tc.alloc_tile_pool(name="p_scp", bufs=2, space="PSUM")
        p_pp = tc.alloc_tile_pool(name="p_pp", bufs=1, space="PSUM")
        p_psD = tc.alloc_tile_pool(name="p_psD", bufs=2, space="PSUM")

        # wo in fp8 DoubleRow layout: [contract-sub 128, head-pair 2, cols],
        # host-prescaled x16 (weights at sigma=0.02 would be e4m3-subnormal);
        # cen is prescaled x16 through the rsv bias, drains scale by 1/256.
        wo8 = p_wO.tile([P, 2, 2048], FP8, name="wo8", tag="wo8")
        nc.sync.dma_start(out=wo8[:], in_=wo8_d[:])
        # Lsgn [P, 1] const: +1 at the time row (VSP), -1 elsewhere, so one
        # matmul against sq gives innr = t^2 - sum(space^2) directly.
        Lsgn = p_wO.tile([P, 1], F32, name="L2", tag="L2")
        nc.sync.dma_start(out=Lsgn[:], in_=l2_d[:])

        def sc_pair(g, h, jp, jmax):
            # scores for j-tiles jp, jp+1 into one [P, 2, NCH] fp16 (1-bank)
            # tile. Diagonal tiles compute full 512 q cols (masked cols are
            # real scores, zeroed in ex after the exp).
            c0 = g * NCH
            sc = p_scp.tile([P, 2, NCH], F32, name="sc", tag="sc", bufs=2)
            for dj in range(2):
                j = jp + dj
                nc.tensor.matmul(sc[:, dj, :], ksA[h][:, j * P:(j + 1) * P],
                                 qsA[h][:, c0:c0 + NCH], start=True, stop=False)
                nc.tensor.matmul(sc[:, dj, :], ksB[h][:, j * P:(j + 1) * P],
                                 qsB[h][:, c0:c0 + NCH], start=False, stop=True)
            return sc

        def exp_av(g, h, jp, ave, sc, jmax):
            ex = p_ex.tile([P, 2, NCH], BF16, name="ex", tag="ex", bufs=3)
            nc.scalar.activation(ex[:], sc[:], AF.Exp, scale=exp_scale)
            if causal:
                for dj in range(2):
                    j = jp + dj
                    d = j - g * GQ
                    if d >= 0:
                        if d > 0:
                            nc.vector.memset(ex[:, dj, 0:d * P], 0.0)
                        nc.vector.tensor_mul(ex[:, dj, d * P:(d + 1) * P],
                                             ex[:, dj, d * P:(d + 1) * P],
                                             diagmask[:])
            for dj in range(2):
                j = jp + dj
                nc.tensor.matmul(ave[:], Vp[h][:, j, :], ex[:, dj, :],
                                 start=(j == 0), stop=(j == jmax - 1))

        def epilogue(g, h, ave, cen2):
            # aveT [d, q]: innr per q col = t^2 - sum_d(space^2) > 0 (row VSP
            # is the time coord); one signed-sum matmul against Lsgn gives it
            # directly. rsv = 1/sqrt(innr) is broadcast to 128 partitions by
            # a 1-row outer-product matmul (engines reject stride-0 APs).
            sq = p_cw.tile([P, NCH], F32, name="sq", tag="sq", bufs=2)
            nc.scalar.square(sq[:], ave[:])
            # one PSUM bank reused: innr lands in row 0, then the broadcast
            # matmul overwrites the whole bank after the Ln has consumed it
            ppb = p_pp.tile([P, NCH], F32, name="ppb", tag="ppb", bufs=1)
            nc.tensor.matmul(ppb[0:1, :], Lsgn[:], sq[:], start=True, stop=True)
            lnr = p_cw.tile([1, NCH], F32, name="lnr", tag="lnr", bufs=2)
            nc.scalar.activation(lnr[:], ppb[0:1, :], AF.Ln)
            # rsv carries a x16 prescale (bias = ln 16) so the fp8 cenT slab
            # sits in e4m3's normal range; the wo drain scales by 1/256
            rsv = p_cw.tile([1, NCH], F32, name="rsv", tag="rsv", bufs=2)
            nc.scalar.activation(rsv[:], lnr[:], AF.Exp, scale=-0.5,
                                 bias=ln16_b[0:1, :])
            nc.tensor.matmul(ppb[:], ones_row[:], rsv[:], start=True,
                             stop=True)
            rbs = p_cw.tile([P, NCH], BF16, name="rbs", tag="rbs", bufs=2)
            nc.vector.tensor_copy(rbs[:], ppb[:])
            nc.vector.tensor_mul(cen2[:, h, :], ave[:], rbs[:])

        # wo work is queued as (m, n) jobs and SPRINKLED between the score
        # matmuls and the exp-gated AV matmul: the PE queue then always holds
        # independent work ahead of the semaphore-waiting AV, so the engine
        # never micro-idles (micro-gaps hold the HAM clock gate at half rate).
        wo_jobs = []

        def wo_one(g, cen2, t, n, drain_scalar=False):
            m = g * GQ + t
            n0 = n * NCH
            nn = min(NCH, OUT_COLS - n0)
            ps = p_psD.tile([P, NCH], F32, name="psd", tag="psd", bufs=2)
            # fp8 DoubleRow: both heads' 128-row contractions in one matmul
            nc.tensor.matmul(ps[:, :nn], cen2[:, :, t * P:(t + 1) * P],
                             wo8[:, :, n0:n0 + nn], start=True, stop=True,
                             perf_mode=mybir.MatmulPerfMode.DoubleRow)
            # drains stay off the scalar engine while the attention loop runs
            # (it gates the exp -> AV chain); the final flush alternates onto
            # the then-idle scalar engine. The 1/256 undoes the two x16
            # fp8 prescales (cen and wo).
            ot = p_osb.tile([P, NCH], BF16, name="ot", tag="ot", bufs=6)
            if drain_scalar:
                nc.scalar.mul(ot[:, :nn], ps[:, :nn], 1.0 / 256.0)
            else:
                nc.vector.tensor_scalar_mul(ot[:, :nn], ps[:, :nn], 1.0 / 256.0)
            nc.sync.dma_start(out=out_d[m * P:(m + 1) * P, n0:n0 + nn],
                              in_=ot[:, :nn])

        def wo_emit(k=1):
            for _ in range(k):
                if wo_jobs:
                    wo_jobs.pop(0)()

        prev_cen = None
        for g in range(NG):
            cen2 = p_cw.tile([P, 2, NCH], FP8, name="cen2", tag="cen2", bufs=2)
            for h in range(HPC):
                ave = p_ave.tile([P, NCH], F32, name="ave", tag="ave", bufs=1)
                jmax = (g * GQ + GQ) if causal else NQT
                pend = []
                for jp in range(0, jmax, 2):
                    pend.append((jp, sc_pair(g, h, jp, jmax)))
                    wo_emit()
                    if len(pend) > 1:
                        pj, psc = pend.pop(0)
                        exp_av(g, h, pj, ave, psc, jmax)
                for (pj, psc) in pend:
                    wo_emit()
                    exp_av(g, h, pj, ave, psc, jmax)
                epilogue(g, h, ave, cen2)
                wo_emit(2)
                if h == 0 and prev_cen is not None:
                    cp = prev_cen
                    wo_jobs.extend(
                        (lambda t=t, n=n, cp=cp, gg=g - 1: wo_one(gg, cp, t, n))
                        for t in range(GQ) for n in range(4))
            # drain leftovers before the next group's epilogue can wrap the
            # cen2 double-buffer ring
            wo_emit(len(wo_jobs))
            prev_cen = cen2
        for t in range(GQ):
            for n in range(4):
                wo_one(NG - 1, prev_cen, t, n, drain_scalar=(n % 2 == 1))

        p_psD.release()
        p_pp.release()
        p_scp.release()
        p_ave.release()
        p_osb.release()
        p_wO.release()
        p_cw.release()
        p_ex.release()

        big2.release()
        big.release()
        const.release()

    nc.compile()
    return nc


_CACHE = {}


def _get_program(exp_scale: float, causal: bool):
    key = (round(float(exp_scale), 12), causal)
    if key not in _CACHE:
        _CACHE[key] = _build_program(float(exp_scale), causal)
    return _CACHE[key]


def _rope_perm():
    """Even rope dims first, then odd (host-side column permutation)."""
    return np.concatenate([np.arange(0, RSP, 2), np.arange(1, RSP, 2)])


def kernel(x, start_pos, freqs_cos, freqs_sin, mask, wq_w, wkv_a_w, kv_norm_w,
           wkv_b_w, wo_w, softmax_scale, bias_p, _want_trace=False):
    x2 = np.ascontiguousarray(np.asarray(x, np.float32).reshape(S, DIM))
    xT = np.ascontiguousarray(x2.T)
    wq_w = np.asarray(wq_w, np.float32)
    wkv_a_w = np.asarray(wkv_a_w, np.float32)
    kv_norm_w = np.asarray(kv_norm_w, np.float32)
    wkv_b_w = np.asarray(wkv_b_w, np.float32)
    wo_w = np.asarray(wo_w, np.float32)
    cosT = np.asarray(freqs_cos, np.float32).T
    sinT = np.asarray(freqs_sin, np.float32).T
    cosT = np.ascontiguousarray(
        np.concatenate([cosT, cosT], axis=0).astype(np.float16))
    sinT = np.ascontiguousarray(
        np.concatenate([sinT, sinT], axis=0).astype(np.float16))

    mask = np.asarray(mask)
    causal = bool(np.array_equal(mask, np.triu(np.ones((S, S), bool), k=1)))
    if not causal:
        assert not mask.any(), "only causal or empty masks are supported"

    smax = float(np.asarray(softmax_scale).reshape(-1)[0])
    exp_scale = 2.0 / smax

    rp = _rope_perm()
    # wq per core-pair layout: [nope_h0 | nope_h1 | rope_h0(ev,od) | rope_h1(ev,od)]
    wq_r = wq_w.reshape(DIM, 16, QH)
    wq_nope = wq_r[:, :, :NOPE]                       # (DIM, 16, 128)
    wq_rope = wq_r[:, :, NOPE:][:, :, rp]             # (DIM, 16, 64) permuted
    wq_cores = []
    for c in range(N_CORES):
        h0, h1 = 2 * c, 2 * c + 1
        wq_cores.append(np.concatenate(
            [wq_nope[:, h0], wq_nope[:, h1], wq_rope[:, h0], wq_rope[:, h1]],
            axis=1))
    # wkva: [kv | rope-even | rope-odd]
    wkva_p = wkv_a_w.copy()
    wkva_p[:, KV_RANK:] = wkva_p[:, KV_RANK:][:, rp]
    # wkvb: kvn rows first, time row last
    wkvb_p = np.ascontiguousarray(np.concatenate([wkv_b_w[1:], wkv_b_w[:1]], axis=0))
    wnormT = np.ascontiguousarray(kv_norm_w.reshape(4, P).T)
    # wo rows per head: [v space (1..127), time (0)]
    wo_p = wo_w.reshape(16, P, OUT_COLS)
    wo_p = np.concatenate([wo_p[:, 1:, :], wo_p[:, 0:1, :]], axis=1)
    wo_p = wo_p.reshape(16 * P, OUT_COLS)
    # fp8 DoubleRow layout per core: [128, 2 (head), 2048], x16 prescale
    # (wo entries at sigma=0.02 would land in e4m3's subnormal range)
    wo8_cores = []
    for c in range(N_CORES):
        blk = wo_p[c * WO_ROWS:(c + 1) * WO_ROWS, :] * 16.0
        w8 = np.zeros((P, 2, 2048), np.float32)
        w8[:, 0, :OUT_COLS] = blk[0:P]
        w8[:, 1, :OUT_COLS] = blk[P:WO_ROWS]
        wo8_cores.append(np.ascontiguousarray(
            w8.astype(ml_dtypes.float8_e4m3)))

    nc = _get_program(exp_scale, causal)

    l2c = np.full((P, 1), -1.0, np.float32)
    l2c[VSP, 0] = 1.0

    # one-hot column selectors for the batched [8, 512] time-row reductions:
    # cols 0-63: slab r -> col r ones (full 128 rows), for bsq/qbsq (r=h*4+n)
    # cols 64-95: slab n -> cols {n, 4+n} ones on rows 0-63, for kpsq
    # cols 96-127: slab n -> col n on rows 0-63, col 4+n on rows 64-127 (qsq)
    lcc = np.zeros((P, 128), np.float16)
    for r in range(8):
        lcc[:, 8 * r + r] = 1.0
    for n4 in range(4):
        lcc[0:RSP, 64 + 8 * n4 + n4] = 1.0
        lcc[0:RSP, 64 + 8 * n4 + 4 + n4] = 1.0
        lcc[0:RSP, 96 + 8 * n4 + n4] = 1.0
        lcc[RSP:P, 96 + 8 * n4 + 4 + n4] = 1.0

    xT_bf = np.ascontiguousarray(xT.astype(np.float16))
    wkva_bf = np.ascontiguousarray(wkva_p.astype(np.float16))

    in_maps = []
    for c in range(N_CORES):
        in_maps.append({
            "xT": xT_bf,
            "wq": np.ascontiguousarray(wq_cores[c].astype(np.float16)),
            "wkva": wkva_bf,
            "wnormT": wnormT,
            "wkvb": np.ascontiguousarray(
                wkvb_p[:, c * WB_COLS:(c + 1) * WB_COLS].astype(np.float16)),
            "wo8": wo8_cores[c],
            "cosT": cosT,
            "sinT": sinT,
            "l2c": l2c,
            "lcc": lcc,
            "xsl": np.ascontiguousarray(xT_bf[:, c * SL:(c + 1) * SL]),
            "cossl": np.ascontiguousarray(cosT[:, c * SL:(c + 1) * SL]),
            "sinsl": np.ascontiguousarray(sinT[:, c * SL:(c + 1) * SL]),
        })

    res = run_bass_kernel_spmd(nc, in_maps, core_ids=list(range(N_CORES)),
                               trace=_want_trace)
    kernel.last_result = res

    total = res.results[0]["out"].astype(np.float32)
    for c in range(1, N_CORES):
        total = total + res.results[c]["out"].astype(np.float32)
    t = np.sqrt(np.sum(total * total, axis=-1, keepdims=True) + 1.0)
    out = np.concatenate([t, total], axis=-1)
    return out.reshape(1, S, DIM).astype(np.float32)



# revision 59
# speedup vs baseline: 1.0025x; 1.0025x over previous
"""Lorentz MLA attention kernel for Trainium2, sharded over 8 NeuronCores.

Sharding: tensor-parallel over the 16 attention heads (2 heads per core);
the kv_lora latent projection (wkv_a + RMS norm) is sequence-sharded and
AllGathered. The output projection wo is row-parallel: each core produces a
partial (2048, 2047) output in bf16; the host sums the 8 partials in f32 and
applies the final Lorentz lift.

Device-side layout notes:
- Everything flows transposed ([feature, s]) so all matmuls contract on the
  partition axis without transposing x on device (host passes x^T).
- The 193-dim Lorentz q/k contraction is chunked [128 nope] + [64 rope + 1
  time]; the time rows sit at partition 64 of the 65-row "B" tiles.
- Rope dims are permuted even-pairs-first on the host so rotary is aligned
  [32, n] ops; weight columns are permuted to match.
- All matmul operands are bf16 (f32r at <256 moving columns runs at 1/4 PE
  rate); PSUM accumulation stays f32.
- Scalar engine uses ONLY the natural_log_exp activation table: every sqrt
  is computed as exp(0.5*ln(1+x)) so no ACT table reloads ever happen.
- Softmax max-pass skipped (scores <= 0 on the hyperboloid) and the softmax
  denominator cancels inside the Lorentz centroid normalization. V' carries
  its time coordinate in column 127 (wo rows are permuted on host to match).
- AV runs Vp-stationary: one N=512 matmul per j-tile accumulates aveT [d, q]
  in PSUM, so the centroid epilogue and the wo projection need no transposes.
  Scores for two j-tiles share one 2-bank PSUM tile; each exp call covers
  1024 columns, halving the scalar engine's per-call pipeline-fill cost.
- q/k time rows are batched: one-hot column-selector matmuls accumulate all
  (head, chunk) |.|^2 sums into an [8, 512] PSUM tile, so one ln/exp pair
  replaces sixteen 1-lane activation calls; squares run on the DVE (dual
  SBUF reads at 2x fp16 rate) instead of the scalar engine.
- The attention pair-loop is software-pipelined, and the previous group's wo
  matmuls are sprinkled between the score and (exp-gated) AV matmuls: the PE
  queue always holds independent work ahead of a semaphore wait, so it never
  micro-idles (micro-gaps hold the HAM clock gate at the 1.2 GHz K=4/8
  state; dense streams keep the 13/16 GPIO-limited 1.95 GHz).
- V' tiles are produced with PE transposes, drained on the scalar engine.
"""

import os
import sys
import types

import numpy as np
import ml_dtypes


def _ensure_axon_hooks():
    """Recreate the missing antenv.axon_hooks module so NTFF tracing works."""
    if "antenv.axon_hooks" in sys.modules:
        return
    try:
        import antenv
        from trn_agent_boot.trn_boot import _ntff_profile_via_ctypes

        hook = _ntff_profile_via_ctypes("/opt/axon/libaxon_pjrt.so")
        mod = types.ModuleType("antenv.axon_hooks")
        mod.get_axon_ntff_profile_hook = lambda: hook
        mod.set_axon_ntff_profile_hook = lambda h: None
        sys.modules["antenv.axon_hooks"] = mod
        antenv.axon_hooks = mod
    except Exception:
        pass


_ensure_axon_hooks()

import concourse.bacc as bacc
import concourse.bass as bass
import concourse.tile as tile
from concourse import mybir
import concourse.bass_utils as bass_utils
from concourse.bass_utils import run_bass_kernel_spmd
from concourse.masks import make_identity, make_upper_triangular

# zero-egress container: make the S3 artifact upload in the profile path a no-op
bass_utils.upload_artifacts = lambda tmpdir: tmpdir

F32 = mybir.dt.float32
BF16 = mybir.dt.float16  # 16-bit compute dtype (fp16: 10 mantissa bits)
FP8 = mybir.dt.float8e4  # e4m3, used for the latent gather payload
AF = mybir.ActivationFunctionType
AX = mybir.AxisListType
ALU = mybir.AluOpType

N_CORES = 8
P = 128
S = 2048          # sequence length
DIM = 2048        # model dim
NDC = DIM // P    # 16 contraction chunks over DIM
NQT = S // P      # 16 q/k tiles of 128
HPC = 2           # heads per core
NOPE = 128
RSP = 64          # rotary space dim
VSP = 127         # v space dim
KV_RANK = 512
EPS_RMS = 1e-6
QH = NOPE + RSP               # 192 q space rows per head
WQ_COLS = HPC * QH            # 384
WB_COLS = HPC * (NOPE + VSP)  # 510
WO_ROWS = HPC * P             # 256
OUT_COLS = DIM - 1            # 2047
NCH = 512                     # column chunk
NA = S // NCH                 # 4 chunks
SL = S // N_CORES             # 256
GR = KV_RANK + RSP + 1        # gathered rows: kvn + kpe + t_row


def _build_program(exp_scale: float, causal: bool):
    nc = bacc.Bacc("TRN2", target_bir_lowering=False, debug=False,
                   num_devices=N_CORES)

    xT_d = nc.dram_tensor("xT", [DIM, S], BF16, kind="ExternalInput")
    wq_d = nc.dram_tensor("wq", [DIM, WQ_COLS], BF16, kind="ExternalInput")
    wkva_d = nc.dram_tensor("wkva", [DIM, KV_RANK + RSP], BF16, kind="ExternalInput")
    wnormT_d = nc.dram_tensor("wnormT", [P, 4], F32, kind="ExternalInput")
    wkvb_d = nc.dram_tensor("wkvb", [KV_RANK + 1, WB_COLS], BF16, kind="ExternalInput")
    wo_d = nc.dram_tensor("wo", [WO_ROWS, OUT_COLS], BF16, kind="ExternalInput")
    cosT_d = nc.dram_tensor("cosT", [RSP, S], BF16, kind="ExternalInput")
    sinT_d = nc.dram_tensor("sinT", [RSP, S], BF16, kind="ExternalInput")
    l2_d = nc.dram_tensor("l2c", [P, 1], F32, kind="ExternalInput")
    lcc_d = nc.dram_tensor("lcc", [P, 128], BF16, kind="ExternalInput")
    out_d = nc.dram_tensor("out", [S, OUT_COLS], BF16, kind="ExternalOutput")
    xsl_d = nc.dram_tensor("xsl", [DIM, SL], BF16, kind="ExternalInput")
    cossl_d = nc.dram_tensor("cossl", [RSP, SL], BF16, kind="ExternalInput")
    sinsl_d = nc.dram_tensor("sinsl", [RSP, SL], BF16, kind="ExternalInput")
    gin = nc.dram_tensor("gin", [GR, SL], BF16)
    gout = nc.dram_tensor("gout", [N_CORES, GR, SL], BF16, addr_space="Shared")

    with tile.TileContext(nc) as tc:
        if os.environ.get("LMLA_NO_TABLE_PRELOAD") != "1":
            # Preload the combined ln+exp ACT table once; without this the
            # auto-placement pass alternates exp->table0 / ln->table5 loads
            # (1.28us each) all through the attention loop.
            nc.scalar.add_instruction(mybir.InstLoadActFuncSet(
                name=nc.get_next_instruction_name(), act_func_set_id=6,
                ins=[], outs=[]))
        const = tc.alloc_tile_pool(name="const", bufs=1)
        identity = const.tile([P, P], BF16)
        make_identity(nc, identity)
        diagmask = const.tile([P, P], BF16)
        make_upper_triangular(nc, diagmask, val=1.0, diag=True)
        wnormT = const.tile([P, 4], F32)
        nc.sync.dma_start(out=wnormT[:], in_=wnormT_d[:])
        Lt = const.tile([P, 4, 2], BF16)  # [ones | wnorm^2] per latent chunk
        for c in range(4):
            nc.vector.memset(Lt[:, c, 0:1], 1.0)
            nc.vector.tensor_mul(Lt[:, c, 1:2], wnormT[:, c:c + 1], wnormT[:, c:c + 1])
        ones_col = const.tile([P, 1], BF16)
        nc.vector.memset(ones_col[:], 1.0)
        ones_row = const.tile([1, P], F32)
        nc.vector.memset(ones_row[:], 1.0)
        ones_row_bf = const.tile([1, P], BF16)
        nc.vector.memset(ones_row_bf[:], 1.0)
        eps_b = const.tile([P, 1], F32)
        nc.vector.memset(eps_b[:], EPS_RMS)
        ln16_b = const.tile([P, 1], F32)
        nc.vector.memset(ln16_b[:], 2.772588722239781)

        # Long-lived tiles.
        big = tc.alloc_tile_pool(name="big", bufs=1)
        qsA = [big.tile([P, S], BF16, name=f"qsA_{h}", tag=f"qsA_{h}") for h in range(HPC)]
        qsB = [big.tile([RSP + 1, S], BF16, name=f"qsB_{h}", tag=f"qsB_{h}") for h in range(HPC)]
        kv = [big.tile([P, S], BF16, name=f"kv_{c}", tag=f"kv_{c}") for c in range(4)]
        kpe = big.tile([RSP, S], BF16, name="kpe", tag="kpe")
        ksB = [big.tile([RSP + 1, S], BF16, name=f"ksB_{h}", tag=f"ksB_{h}") for h in range(HPC)]
        Vp = [big.tile([P, NQT, P], BF16, name=f"Vp_{h}", tag=f"Vp_{h}") for h in range(HPC)]
        t_row_bf = big.tile([1, S], BF16, name="t_row_bf", tag="t_row_bf")

        # ------------- Slice phase: kv latent on this core's s-slice ---------
        p_wKV = tc.alloc_tile_pool(name="p_wKV", bufs=1)
        p_sl = tc.alloc_tile_pool(name="p_sl", bufs=1)
        p_pssl = tc.alloc_tile_pool(name="p_pssl", bufs=2, space="PSUM")
        wKV = []
        for dc in range(NDC):
            w = p_wKV.tile([P, KV_RANK + RSP], BF16, name=f"wKV_{dc}", tag=f"wKV_{dc}")
            nc.sync.dma_start(out=w[:], in_=wkva_d[dc * P:(dc + 1) * P, :])
            wKV.append(w)
        xsl_t = p_sl.tile([P, NDC, SL], BF16, name="xsl_t", tag="xsl_t")
        for dc in range(NDC):
            nc.sync.dma_start(out=xsl_t[:, dc, :],
                              in_=xsl_d[dc * P:(dc + 1) * P, :])
        cossl = p_sl.tile([RSP, SL], BF16, name="cossl", tag="cossl")
        sinsl = p_sl.tile([RSP, SL], BF16, name="sinsl", tag="sinsl")
        nc.sync.dma_start(out=cossl[:], in_=cossl_d[:])
        nc.sync.dma_start(out=sinsl[:], in_=sinsl_d[:])

        # phase-A weights prefetch during the slice compute (pure loads, no
        # waits, so they issue immediately on sync)
        p_wA = tc.alloc_tile_pool(name="p_wA", bufs=1)
        p_qsc = tc.alloc_tile_pool(name="p_qsc", bufs=1)
        cosT = p_qsc.tile([RSP, S], BF16, name="cosT", tag="cosT")
        sinT = p_qsc.tile([RSP, S], BF16, name="sinT", tag="sinT")
        nc.sync.dma_start(out=cosT[:], in_=cosT_d[:])
        nc.sync.dma_start(out=sinT[:], in_=sinT_d[:])
        wQ = []
        for dc in range(NDC):
            w = p_wA.tile([P, WQ_COLS], BF16, name=f"wQ_{dc}", tag=f"wQ_{dc}")
            nc.sync.dma_start(out=w[:], in_=wq_d[dc * P:(dc + 1) * P, :])
            wQ.append(w)

        kvsl = [p_sl.tile([P, SL], F32, name=f"kvsl_{c}", tag=f"kvsl_{c}")
                for c in range(4)]
        kpesl = p_sl.tile([RSP, SL], F32, name="kpesl", tag="kpesl")
        for c in range(4):
            ps = p_pssl.tile([P, SL], F32, name="psl", tag="psl", bufs=2)
            for dc in range(NDC):
                nc.tensor.matmul(ps[:], wKV[dc][:, c * P:(c + 1) * P],
                                 xsl_t[:, dc, :], start=(dc == 0), stop=(dc == NDC - 1))
            nc.vector.tensor_copy(kvsl[c][:], ps[:])
        ps = p_pssl.tile([P, SL], F32, name="psl", tag="psl", bufs=2)
        for dc in range(NDC):
            nc.tensor.matmul(ps[:RSP, :], wKV[dc][:, KV_RANK:],
                             xsl_t[:, dc, :], start=(dc == 0), stop=(dc == NDC - 1))
        nc.vector.tensor_copy(kpesl[:], ps[:RSP, :])

        # RMS stats on the slice
        ps_s = p_pssl.tile([1, SL], F32, name="ps_s", tag="ps_s", bufs=1)
        ps_w = p_pssl.tile([1, SL], F32, name="ps_w", tag="ps_w", bufs=1)
        for c in range(4):
            ksq = p_sl.tile([P, SL], BF16, name="ksq", tag="ksq", bufs=2)
            nc.scalar.square(ksq[:], kvsl[c][:])
            nc.tensor.matmul(ps_s[:], Lt[:, c, 0:1], ksq[:], start=(c == 0), stop=(c == 3))
            nc.tensor.matmul(ps_w[:], Lt[:, c, 1:2], ksq[:], start=(c == 0), stop=(c == 3))
        # inv_rms = exp(-0.5 * ln(mean_sq + eps)) ; single ACT table (ln/exp)
        ln_s = p_sl.tile([1, SL], F32, name="ln_s", tag="ln_s")
        nc.scalar.activation(ln_s[:], ps_s[:], AF.Ln, bias=eps_b[0:1, :],
                             scale=1.0 / KV_RANK)
        inv_rms = p_sl.tile([1, SL], F32, name="inv_rms", tag="inv_rms")
        nc.scalar.activation(inv_rms[:], ln_s[:], AF.Exp, scale=-0.5)
        tmp_r = p_sl.tile([1, SL], F32, name="tmp_r", tag="tmp_r")
        nc.vector.tensor_copy(tmp_r[:], ps_w[:])
        nc.vector.tensor_mul(tmp_r[:], tmp_r[:], inv_rms[:])
        nc.vector.tensor_mul(tmp_r[:], tmp_r[:], inv_rms[:])
        t_ln = p_sl.tile([1, SL], F32, name="t_ln", tag="t_ln")
        nc.scalar.activation(t_ln[:], tmp_r[:], AF.Ln, bias=1.0)
        t_st = p_sl.tile([1, SL], BF16, name="t_st", tag="t_st")
        nc.scalar.activation(t_st[:], t_ln[:], AF.Exp, scale=0.5)

        # broadcast inv_rms via outer product; fused scale -> bf16 stage
        rb = p_pssl.tile([P, SL], F32, name="rb", tag="rb", bufs=1)
        nc.tensor.matmul(rb[:], ones_row[:], inv_rms[:], start=True, stop=True)
        kvn_st = [p_sl.tile([P, SL], BF16, name=f"kvn_st_{c}", tag=f"kvn_st_{c}")
                  for c in range(4)]
        for c in range(4):
            nc.vector.scalar_tensor_tensor(
                kvn_st[c][:], kvsl[c][:], wnormT[:, c:c + 1], rb[:],
                op0=ALU.mult, op1=ALU.mult)

        # rotary on the k_pe slice
        rtl = p_sl.tile([RSP, SL], F32, name="rtl", tag="rtl")
        kpe_st = p_sl.tile([RSP, SL], BF16, name="kpe_st", tag="kpe_st")
        x0 = kpesl[0:32, :]
        x1 = kpesl[32:64, :]
        nc.vector.tensor_mul(rtl[32:64, :], x0, sinsl[0:32, :])
        nc.vector.tensor_mul(rtl[0:32, :], x1, sinsl[32:64, :])
        nc.vector.tensor_mul(x0, x0, cossl[0:32, :])
        nc.vector.tensor_mul(x1, x1, cossl[32:64, :])
        nc.vector.tensor_sub(kpe_st[0:32, :], x0, rtl[0:32, :])
        nc.vector.tensor_add(kpe_st[32:64, :], x1, rtl[32:64, :])

        # ship slice, gather full (single bf16 payload). The gin writes go on
        # the vector queue (their producers): on the in-order sync queue they
        # would block the phase-A weight/x DMA issues behind the slice tail.
        for c in range(4):
            nc.gpsimd.dma_start(out=gin[c * P:(c + 1) * P, :], in_=kvn_st[c][:])
        nc.gpsimd.dma_start(out=gin[KV_RANK:KV_RANK + RSP, :], in_=kpe_st[:])
        nc.gpsimd.dma_start(out=gin[KV_RANK + RSP:, :], in_=t_st[:])
        nc.gpsimd.collective_compute(
            "AllGather", ALU.bypass,
            replica_groups=[list(range(N_CORES))],
            ins=[gin[:]], outs=[gout[:]])
        # keep p_sl/p_wKV alive through phase A: recycling their SBUF for the
        # xt tiles makes the PE wait on the slice tail + gin DMA reads
        p_pssl.release()

        # --- Phase A: q projection over the full sequence --------------------
        # weight col layout (host): [qnope0 | qnope1 | qrope0(ev,od) | qrope1]
        # Per n-chunk: matmuls, drains to bf16, rotary (vector). q/k/v time
        # rows are all computed in phase B from the bf16 SBUF copies (DVE
        # squares + batched [8, 512] ln/exp).
        p_xs = tc.alloc_tile_pool(name="p_xs", bufs=1)
        p_psA = tc.alloc_tile_pool(name="p_psA", bufs=3, space="PSUM")

        for n in range(NA):
            n0 = n * NCH
            xt = p_xs.tile([P, NDC, NCH], BF16, name="xt", tag="xt", bufs=2)
            src = xT_d[:, n0:n0 + NCH].rearrange("(dc p) s -> p dc s", p=P)
            for dc in range(NDC):
                nc.sync.dma_start(out=xt[:, dc, :], in_=src[:, dc, :])

            # rope chunk for both heads: rows [h0ev|h0od|h1ev|h1od]
            ps = p_psA.tile([P, NCH], F32, name="psa", tag="psa", bufs=3)
            for dc in range(NDC):
                nc.tensor.matmul(ps[:], wQ[dc][:, 2 * P:3 * P], xt[:, dc, :],
                                 start=(dc == 0), stop=(dc == NDC - 1))
            for h in range(HPC):
                nc.scalar.copy(qsB[h][0:RSP, n0:n0 + NCH], ps[h * RSP:(h + 1) * RSP, :])
            # rotary, in place on bf16 (2x DVE mode)
            rt = p_qsc.tile([RSP, NCH], BF16, name="rt", tag="rt", bufs=2)
            for h in range(HPC):
                gx0 = qsB[h][0:32, n0:n0 + NCH]
                gx1 = qsB[h][32:64, n0:n0 + NCH]
                nc.vector.tensor_mul(rt[32:64, :], gx0, sinT[0:32, n0:n0 + NCH])
                nc.vector.tensor_mul(rt[0:32, :], gx1, sinT[32:64, n0:n0 + NCH])
                nc.vector.tensor_mul(gx0, gx0, cosT[0:32, n0:n0 + NCH])
                nc.vector.tensor_mul(gx1, gx1, cosT[32:64, n0:n0 + NCH])
                nc.vector.tensor_sub(gx0, gx0, rt[0:32, :])
                nc.vector.tensor_add(gx1, gx1, rt[32:64, :])

            for h in range(HPC):
                ps = p_psA.tile([P, NCH], F32, name="psa", tag="psa", bufs=3)
                for dc in range(NDC):
                    nc.tensor.matmul(ps[:], wQ[dc][:, h * P:(h + 1) * P],
                                     xt[:, dc, :], start=(dc == 0), stop=(dc == NDC - 1))
                nc.vector.tensor_copy(qsA[h][:, n0:n0 + NCH], ps[:])
        p_psA.release()
        p_xs.release()
        p_qsc.release()
        p_wA.release()
        p_sl.release()
        p_wKV.release()

        # gather unpack. Emitted AFTER the phase-A pool releases and on the
        # (idle) gpsimd engine: the triggers wait on the collective, so on the
        # in-order sync engine they'd starve phase A's xt loads, and if they
        # precede the releases the release drain (queued behind them on
        # gpsimd) gates every post-release allocation.
        # kv unpack split by 512-col output chunk (k-slot pairs) so phase B's
        # first chunk matmuls fire as soon as their slice of the gather lands
        for nn in range(NA):
            for c in range(4):
                nc.gpsimd.dma_start(
                    out=kv[c][:, nn * NCH:(nn + 1) * NCH].rearrange(
                        "p (k s) -> p k s", k=2),
                    in_=gout[2 * nn:2 * nn + 2, c * P:(c + 1) * P, :].rearrange(
                        "k p s -> p k s"))
        # k_pe rows are shared by both heads: unpack straight into both ksB
        # tiles (and once more into kpe for the k-time squares)
        for h in range(HPC):
            nc.gpsimd.dma_start(
                out=ksB[h][0:RSP, :].rearrange("p (k s) -> p k s", k=N_CORES),
                in_=gout[:, KV_RANK:KV_RANK + RSP, :].rearrange("k p s -> p k s"))
        nc.gpsimd.dma_start(
            out=kpe[:].rearrange("p (k s) -> p k s", k=N_CORES),
            in_=gout[:, KV_RANK:KV_RANK + RSP, :].rearrange("k p s -> p k s"))
        nc.gpsimd.dma_start(
            out=t_row_bf[:].rearrange("p (k s) -> p k s", k=N_CORES),
            in_=gout[:, KV_RANK + RSP:, :].rearrange("k p s -> p k s"))

        # --- Phase B: kv_b projection + k/v time rows + V' assembly ----------
        big2 = tc.alloc_tile_pool(name="big2", bufs=1)
        p_wB = tc.alloc_tile_pool(name="p_wB", bufs=1)
        p_psB = tc.alloc_tile_pool(name="p_psB", bufs=3, space="PSUM")
        p_pkv = tc.alloc_tile_pool(name="p_pkv", bufs=2, space="PSUM")
        p_ptv = tc.alloc_tile_pool(name="p_ptv", bufs=2, space="PSUM")
        p_bsc = tc.alloc_tile_pool(name="p_bsc", bufs=1)
        wb_k = []
        for k in range(4):
            w = p_wB.tile([P, WB_COLS], BF16, name=f"wbk_{k}", tag=f"wbk_{k}")
            nc.sync.dma_start(out=w[:], in_=wkvb_d[k * P:(k + 1) * P, :])
            wb_k.append(w)
        wb_t = p_wB.tile([1, WB_COLS], BF16, name="wb_t", tag="wb_t")
        nc.sync.dma_start(out=wb_t[:], in_=wkvb_d[KV_RANK:KV_RANK + 1, :])

        ksA = [big2.tile([P, S], BF16, name=f"ksA_{h}", tag=f"ksA_{h}") for h in range(HPC)]
        vts = [big2.tile([P, S], BF16, name=f"vts_{h}", tag=f"vts_{h}") for h in range(HPC)]

        def kvb_mms(ps, col0, msize, n0):
            for k in range(4):
                nc.tensor.matmul(ps[:msize, :], wb_k[k][:, col0:col0 + msize],
                                 kv[k][:, n0:n0 + NCH], start=(k == 0), stop=False)
            nc.tensor.matmul(ps[:msize, :], wb_t[:, col0:col0 + msize],
                             t_row_bf[:, n0:n0 + NCH], start=False, stop=True)

        # batched time-row accumulators: rows r = h*4 + n of [8, NCH]; a
        # single ln/exp pair then covers all (h, n) at once. The one-hot
        # column selectors come from the host (lcc): zero columns write
        # zeros to the other rows, which is harmless under accumulation.
        lcc = p_wB.tile([P, 128], BF16, name="lcc", tag="lcc")
        nc.sync.dma_start(out=lcc[:], in_=lcc_d[:])
        qkall = p_pkv.tile([8, NCH], F32, name="qkall", tag="qkall", bufs=1)
        pkall = p_pkv.tile([8, NCH], F32, name="pkall", tag="pkall", bufs=1)

        for n in range(NA):
            n0 = n * NCH
            # DVE squares from the bf16 SBUF copies (dual SBUF reads, 2x rate)
            kpsq = p_bsc.tile([RSP, NCH], BF16, name="kpsq", tag="kpsq", bufs=2)
            nc.vector.tensor_mul(kpsq[:], kpe[:, n0:n0 + NCH],
                                 kpe[:, n0:n0 + NCH])
            qsq = p_bsc.tile([P, NCH], BF16, name="qsq", tag="qsq", bufs=2)
            for h in range(HPC):
                nc.vector.tensor_mul(qsq[h * RSP:(h + 1) * RSP, :],
                                     qsB[h][0:RSP, n0:n0 + NCH],
                                     qsB[h][0:RSP, n0:n0 + NCH])
            nc.tensor.matmul(qkall[:], lcc[:, 96 + 8 * n:96 + 8 * n + 8],
                             qsq[:], start=(n == 0), stop=False,
                             skip_group_check=True)
            for h in range(HPC):
                r = h * 4 + n
                qbsq = p_bsc.tile([P, NCH], BF16, name="qbsq", tag="qbsq", bufs=2)
                nc.vector.tensor_mul(qbsq[:], qsA[h][:, n0:n0 + NCH],
                                     qsA[h][:, n0:n0 + NCH])
                nc.tensor.matmul(qkall[:], lcc[:, 8 * r:8 * r + 8], qbsq[:],
                                 start=False, stop=(n == NA - 1 and h == HPC - 1),
                                 skip_group_check=True)
            for h in range(HPC):
                c0 = h * (NOPE + VSP)
                r = h * 4 + n
                # k_nope
                ps = p_psB.tile([P, NCH], F32, name="psb", tag="psb", bufs=3)
                kvb_mms(ps, c0, NOPE, n0)
                nc.vector.tensor_copy(ksA[h][:, n0:n0 + NCH], ps[:])
                bsq = p_bsc.tile([P, NCH], BF16, name="bsq", tag="bsq", bufs=2)
                nc.vector.tensor_mul(bsq[:], ksA[h][:, n0:n0 + NCH],
                                     ksA[h][:, n0:n0 + NCH])
                nc.tensor.matmul(pkall[:], lcc[:, 8 * r:8 * r + 8], bsq[:],
                                 start=(n == 0 and h == 0), stop=False,
                                 skip_group_check=True)
                if h == HPC - 1:
                    nc.tensor.matmul(pkall[:], lcc[0:RSP, 64 + 8 * n:64 + 8 * n + 8],
                                     kpsq[:],
                                     start=False, stop=(n == NA - 1),
                                     skip_group_check=True)
                # v (127 space rows; time goes in row 127 of vts)
                ps = p_psB.tile([P, NCH], F32, name="psb", tag="psb", bufs=3)
                kvb_mms(ps, c0 + NOPE, VSP, n0)
                nc.vector.tensor_copy(vts[h][0:VSP, n0:n0 + NCH], ps[:VSP, :])
                vsq = p_bsc.tile([VSP, NCH], BF16, name="vsq", tag="vsq", bufs=2)
                nc.vector.tensor_mul(vsq[:], vts[h][0:VSP, n0:n0 + NCH],
                                     vts[h][0:VSP, n0:n0 + NCH])
                pv = p_pkv.tile([1, NCH], F32, name="pv", tag="pv", bufs=1)
                nc.tensor.matmul(pv[:], ones_col[0:VSP, :], vsq[:],
                                 start=True, stop=True)
                vln = p_bsc.tile([1, NCH], F32, name="vln", tag="vln", bufs=2)
                nc.scalar.activation(vln[:], pv[:], AF.Ln, bias=1.0)
                # engines can't write a region based at partition 127; go via
                # a scratch row + SBUF->SBUF DMA
                vtr = p_bsc.tile([1, NCH], BF16, name="vtr", tag="vtr", bufs=2)
                nc.scalar.activation(vtr[:], vln[:], AF.Exp, scale=0.5)
                nc.sync.dma_start(out=vts[h][VSP:VSP + 1, n0:n0 + NCH],
                                  in_=vtr[:])
                # V' tiles for this chunk: PE transposes (DMA xbar transposes
                # get scheduled lazily, serialize against other DMAs, and
                # stall the attention loop's AV matmuls)
                for j in range(n * 4, n * 4 + 4):
                    tpv = p_ptv.tile([P, P], BF16, name="tpv", tag="tpv", bufs=2)
                    nc.tensor.transpose(tpv[:], vts[h][:, j * P:(j + 1) * P],
                                        identity[:])
                    nc.scalar.copy(Vp[h][:, j, :], tpv[:])
        # finalize the time rows: one ln/exp pair per quantity
        kt8 = p_bsc.tile([8, NCH], BF16, name="kt8", tag="kt8")
        kl8 = p_bsc.tile([8, NCH], F32, name="kl8", tag="kl8")
        nc.scalar.activation(kl8[:], pkall[:], AF.Ln, bias=1.0)
        nc.scalar.activation(kt8[:], kl8[:], AF.Exp, scale=0.5)
        qt8 = p_bsc.tile([8, NCH], BF16, name="qt8", tag="qt8")
        ql8 = p_bsc.tile([8, NCH], F32, name="ql8", tag="ql8")
        nc.scalar.activation(ql8[:], qkall[:], AF.Ln, bias=1.0)
        nc.scalar.activation(qt8[:], ql8[:], AF.Exp, scale=0.5)
        qt8n = p_bsc.tile([8, NCH], BF16, name="qt8n", tag="qt8n")
        nc.vector.tensor_scalar_mul(qt8n[:], qt8[:], -1.0)
        for h in range(HPC):
            for n in range(NA):
                r = h * 4 + n
                n0 = n * NCH
                nc.gpsimd.dma_start(out=ksB[h][RSP:RSP + 1, n0:n0 + NCH],
                                    in_=kt8[r:r + 1, :])
                nc.gpsimd.dma_start(out=qsB[h][RSP:RSP + 1, n0:n0 + NCH],
                                    in_=qt8n[r:r + 1, :])
        p_bsc.release()
        p_ptv.release()
        p_pkv.release()
        p_psB.release()
        p_wB.release()

        # ---------------- Phase C: attention ---------------------------------
        # scoresT layout [k, q]. AV runs Vp-stationary: one N=512 matmul per
        # j-tile accumulating aveT [d, q] in PSUM, so the epilogue and the wo
        # projection need no transposes at all. Scores for two j-tiles land in
        # one 2-bank PSUM tile so each exp call covers 1024 columns (the
        # scalar engine's per-call pipeline fill is ~290ns). The pair-loop is
        # software-pipelined two deep so the PE never waits on the exp.
        GQ = NCH // P
        NG = S // NCH
        p_ex = tc.alloc_tile_pool(name="p_ex", bufs=4)
        p_cw = tc.alloc_tile_pool(name="p_cw", bufs=2)
        p_wO = tc.alloc_tile_pool(name="p_wO", bufs=1)
        p_osb = tc.alloc_tile_pool(name="p_osb", bufs=4)
        p_ave = tc.alloc_tile_pool(name="p_ave", bufs=1, space="PSUM")
        p_scp = tc.alloc_tile_pool(name="p_scp", bufs=2, space="PSUM")
        p_pp = tc.alloc_tile_pool(name="p_pp", bufs=1, space="PSUM")
        p_psD = tc.alloc_tile_pool(name="p_psD", bufs=2, space="PSUM")

        wo_sb = []
        for h in range(HPC):
            w = p_wO.tile([P, OUT_COLS], BF16, name=f"wo_{h}", tag=f"wo_{h}")
            nc.sync.dma_start(out=w[:], in_=wo_d[h * P:(h + 1) * P, :])
            wo_sb.append(w)
        # Lsgn [P, 1] const: +1 at the time row (VSP), -1 elsewhere, so one
        # matmul against sq gives innr = t^2 - sum(space^2) directly.
        Lsgn = p_wO.tile([P, 1], F32, name="L2", tag="L2")
        nc.sync.dma_start(out=Lsgn[:], in_=l2_d[:])

        def sc_pair(g, h, jp, jmax):
            # scores for j-tiles jp, jp+1 into one [P, 2, NCH] fp16 (1-bank)
            # tile. Diagonal tiles compute full 512 q cols (masked cols are
            # real scores, zeroed in ex after the exp).
            c0 = g * NCH
            sc = p_scp.tile([P, 2, NCH], F32, name="sc", tag="sc", bufs=2)
            for dj in range(2):
                j = jp + dj
                nc.tensor.matmul(sc[:, dj, :], ksA[h][:, j * P:(j + 1) * P],
                                 qsA[h][:, c0:c0 + NCH], start=True, stop=False)
                nc.tensor.matmul(sc[:, dj, :], ksB[h][:, j * P:(j + 1) * P],
                                 qsB[h][:, c0:c0 + NCH], start=False, stop=True)
            return sc

        def exp_av(g, h, jp, ave, sc, jmax):
            ex = p_ex.tile([P, 2, NCH], BF16, name="ex", tag="ex", bufs=3)
            nc.scalar.activation(ex[:], sc[:], AF.Exp, scale=exp_scale)
            if causal:
                for dj in range(2):
                    j = jp + dj
                    d = j - g * GQ
                    if d >= 0:
                        if d > 0:
                            nc.vector.memset(ex[:, dj, 0:d * P], 0.0)
                        nc.vector.tensor_mul(ex[:, dj, d * P:(d + 1) * P],
                                             ex[:, dj, d * P:(d + 1) * P],
                                             diagmask[:])
            for dj in range(2):
                j = jp + dj
                nc.tensor.matmul(ave[:], Vp[h][:, j, :], ex[:, dj, :],
                                 start=(j == 0), stop=(j == jmax - 1))

        def epilogue(g, h, ave, cen2):
            # aveT [d, q]: innr per q col = t^2 - sum_d(space^2) > 0 (row VSP
            # is the time coord); one signed-sum matmul against Lsgn gives it
            # directly. rsv = 1/sqrt(innr) is broadcast to 128 partitions by
            # a 1-row outer-product matmul (engines reject stride-0 APs).
            sq = p_cw.tile([P, NCH], F32, name="sq", tag="sq", bufs=2)
            nc.scalar.square(sq[:], ave[:])
            # one PSUM bank reused: innr lands in row 0, then the broadcast
            # matmul overwrites the whole bank after the Ln has consumed it
            ppb = p_pp.tile([P, NCH], F32, name="ppb", tag="ppb", bufs=1)
            nc.tensor.matmul(ppb[0:1, :], Lsgn[:], sq[:], start=True, stop=True)
            lnr = p_cw.tile([1, NCH], F32, name="lnr", tag="lnr", bufs=2)
            nc.scalar.activation(lnr[:], ppb[0:1, :], AF.Ln)
            rsv = p_cw.tile([1, NCH], F32, name="rsv", tag="rsv", bufs=2)
            nc.scalar.activation(rsv[:], lnr[:], AF.Exp, scale=-0.5)
            nc.tensor.matmul(ppb[:], ones_row[:], rsv[:], start=True,
                             stop=True)
            rbs = p_cw.tile([P, NCH], BF16, name="rbs", tag="rbs", bufs=2)
            nc.vector.tensor_copy(rbs[:], ppb[:])
            nc.vector.tensor_mul(cen2[:, h, :], ave[:], rbs[:])

        # wo work is queued as (m, n) jobs and SPRINKLED between the score
        # matmuls and the exp-gated AV matmul: the PE queue then always holds
        # independent work ahead of the semaphore-waiting AV, so the engine
        # never micro-idles (micro-gaps hold the HAM clock gate at half rate).
        wo_jobs = []

        def wo_one(g, cen2, t, n, drain_scalar=False):
            m = g * GQ + t
            n0 = n * NCH
            nn = min(NCH, OUT_COLS - n0)
            ps = p_psD.tile([P, NCH], F32, name="psd", tag="psd", bufs=2)
            nc.tensor.matmul(ps[:, :nn], cen2[:, 0, t * P:(t + 1) * P],
                             wo_sb[0][:, n0:n0 + nn], start=True, stop=False)
            nc.tensor.matmul(ps[:, :nn], cen2[:, 1, t * P:(t + 1) * P],
                             wo_sb[1][:, n0:n0 + nn], start=False, stop=True)
            # drains stay off the scalar engine while the attention loop runs
            # (it gates the exp -> AV chain); the final flush alternates onto
            # the then-idle scalar engine. The 1/256 undoes the two x16
            # fp8 prescales (cen and wo).
            ot = p_osb.tile([P, NCH], BF16, name="ot", tag="ot", bufs=6)
            if drain_scalar:
                nc.scalar.copy(ot[:, :nn], ps[:, :nn])
            else:
                nc.vector.tensor_copy(ot[:, :nn], ps[:, :nn])
            nc.sync.dma_start(out=out_d[m * P:(m + 1) * P, n0:n0 + nn],
                              in_=ot[:, :nn])

        def wo_emit(k=1):
            for _ in range(k):
                if wo_jobs:
                    wo_jobs.pop(0)()

        prev_cen = None
        for g in range(NG):
            cen2 = p_cw.tile([P, 2, NCH], BF16, name="cen2", tag="cen2", bufs=2)
            for h in range(HPC):
                ave = p_ave.tile([P, NCH], F32, name="ave", tag="ave", bufs=1)
                jmax = (g * GQ + GQ) if causal else NQT
                pend = []
                for jp in range(0, jmax, 2):
                    pend.append((jp, sc_pair(g, h, jp, jmax)))
                    wo_emit()
                    if len(pend) > 1:
                        pj, psc = pend.pop(0)
                        exp_av(g, h, pj, ave, psc, jmax)
                for (pj, psc) in pend:
                    wo_emit()
                    exp_av(g, h, pj, ave, psc, jmax)
                epilogue(g, h, ave, cen2)
                wo_emit(2)
                if h == 0 and prev_cen is not None:
                    cp = prev_cen
                    wo_jobs.extend(
                        (lambda t=t, n=n, cp=cp, gg=g - 1: wo_one(gg, cp, t, n))
                        for t in range(GQ) for n in range(4))
            # drain leftovers before the next group's epilogue can wrap the
            # cen2 double-buffer ring
            wo_emit(len(wo_jobs))
            prev_cen = cen2
        for t in range(GQ):
            for n in range(4):
                wo_one(NG - 1, prev_cen, t, n, drain_scalar=(n % 2 == 1))

        p_psD.release()
        p_pp.release()
        p_scp.release()
        p_ave.release()
        p_osb.release()
        p_wO.release()
        p_cw.release()
        p_ex.release()

        big2.release()
        big.release()
        const.release()

    nc.compile()
    return nc


_CACHE = {}


def _get_program(exp_scale: float, causal: bool):
    key = (round(float(exp_scale), 12), causal)
    if key not in _CACHE:
        _CACHE[key] = _build_program(float(exp_scale), causal)
    return _CACHE[key]


def _rope_perm():
    """Even rope dims first, then odd (host-side column permutation)."""
    return np.concatenate([np.arange(0, RSP, 2), np.arange(1, RSP, 2)])


def kernel(x, start_pos, freqs_cos, freqs_sin, mask, wq_w, wkv_a_w, kv_norm_w,
           wkv_b_w, wo_w, softmax_scale, bias_p, _want_trace=False):
    x2 = np.ascontiguousarray(np.asarray(x, np.float32).reshape(S, DIM))
    xT = np.ascontiguousarray(x2.T)
    wq_w = np.asarray(wq_w, np.float32)
    wkv_a_w = np.asarray(wkv_a_w, np.float32)
    kv_norm_w = np.asarray(kv_norm_w, np.float32)
    wkv_b_w = np.asarray(wkv_b_w, np.float32)
    wo_w = np.asarray(wo_w, np.float32)
    cosT = np.asarray(freqs_cos, np.float32).T
    sinT = np.asarray(freqs_sin, np.float32).T
    cosT = np.ascontiguousarray(
        np.concatenate([cosT, cosT], axis=0).astype(np.float16))
    sinT = np.ascontiguousarray(
        np.concatenate([sinT, sinT], axis=0).astype(np.float16))

    mask = np.asarray(mask)
    causal = bool(np.array_equal(mask, np.triu(np.ones((S, S), bool), k=1)))
    if not causal:
        assert not mask.any(), "only causal or empty masks are supported"

    smax = float(np.asarray(softmax_scale).reshape(-1)[0])
    exp_scale = 2.0 / smax

    rp = _rope_perm()
    # wq per core-pair layout: [nope_h0 | nope_h1 | rope_h0(ev,od) | rope_h1(ev,od)]
    wq_r = wq_w.reshape(DIM, 16, QH)
    wq_nope = wq_r[:, :, :NOPE]                       # (DIM, 16, 128)
    wq_rope = wq_r[:, :, NOPE:][:, :, rp]             # (DIM, 16, 64) permuted
    wq_cores = []
    for c in range(N_CORES):
        h0, h1 = 2 * c, 2 * c + 1
        wq_cores.append(np.concatenate(
            [wq_nope[:, h0], wq_nope[:, h1], wq_rope[:, h0], wq_rope[:, h1]],
            axis=1))
    # wkva: [kv | rope-even | rope-odd]
    wkva_p = wkv_a_w.copy()
    wkva_p[:, KV_RANK:] = wkva_p[:, KV_RANK:][:, rp]
    # wkvb: kvn rows first, time row last
    wkvb_p = np.ascontiguousarray(np.concatenate([wkv_b_w[1:], wkv_b_w[:1]], axis=0))
    wnormT = np.ascontiguousarray(kv_norm_w.reshape(4, P).T)
    # wo rows per head: [v space (1..127), time (0)]
    wo_p = wo_w.reshape(16, P, OUT_COLS)
    wo_p = np.concatenate([wo_p[:, 1:, :], wo_p[:, 0:1, :]], axis=1)
    wo_p = wo_p.reshape(16 * P, OUT_COLS)

    nc = _get_program(exp_scale, causal)

    l2c = np.full((P, 1), -1.0, np.float32)
    l2c[VSP, 0] = 1.0

    # one-hot column selectors for the batched [8, 512] time-row reductions:
    # cols 0-63: slab r -> col r ones (full 128 rows), for bsq/qbsq (r=h*4+n)
    # cols 64-95: slab n -> cols {n, 4+n} ones on rows 0-63, for kpsq
    # cols 96-127: slab n -> col n on rows 0-63, col 4+n on rows 64-127 (qsq)
    lcc = np.zeros((P, 128), np.float16)
    for r in range(8):
        lcc[:, 8 * r + r] = 1.0
    for n4 in range(4):
        lcc[0:RSP, 64 + 8 * n4 + n4] = 1.0
        lcc[0:RSP, 64 + 8 * n4 + 4 + n4] = 1.0
        lcc[0:RSP, 96 + 8 * n4 + n4] = 1.0
        lcc[RSP:P, 96 + 8 * n4 + 4 + n4] = 1.0

    xT_bf = np.ascontiguousarray(xT.astype(np.float16))
    wkva_bf = np.ascontiguousarray(wkva_p.astype(np.float16))

    in_maps = []
    for c in range(N_CORES):
        in_maps.append({
            "xT": xT_bf,
            "wq": np.ascontiguousarray(wq_cores[c].astype(np.float16)),
            "wkva": wkva_bf,
            "wnormT": wnormT,
            "wkvb": np.ascontiguousarray(
                wkvb_p[:, c * WB_COLS:(c + 1) * WB_COLS].astype(np.float16)),
            "wo": np.ascontiguousarray(
                wo_p[c * WO_ROWS:(c + 1) * WO_ROWS, :].astype(np.float16)),
            "cosT": cosT,
            "sinT": sinT,
            "l2c": l2c,
            "lcc": lcc,
            "xsl": np.ascontiguousarray(xT_bf[:, c * SL:(c + 1) * SL]),
            "cossl": np.ascontiguousarray(cosT[:, c * SL:(c + 1) * SL]),
            "sinsl": np.ascontiguousarray(sinT[:, c * SL:(c + 1) * SL]),
        })

    res = run_bass_kernel_spmd(nc, in_maps, core_ids=list(range(N_CORES)),
                               trace=_want_trace)
    kernel.last_result = res

    total = res.results[0]["out"].astype(np.float32)
    for c in range(1, N_CORES):
        total = total + res.results[c]["out"].astype(np.float32)
    t = np.sqrt(np.sum(total * total, axis=-1, keepdims=True) + 1.0)
    out = np.concatenate([t, total], axis=-1)
    return out.reshape(1, S, DIM).astype(np.float32)



# revision 63
# speedup vs baseline: 1.0513x; 1.0488x over previous
"""Lorentz MLA attention kernel for Trainium2, sharded over 8 NeuronCores.

Sharding: tensor-parallel over the 16 attention heads (2 heads per core);
the kv_lora latent projection (wkv_a + RMS norm) is sequence-sharded and
AllGathered. The output projection wo is row-parallel: each core produces a
partial (2048, 2047) output in bf16; the host sums the 8 partials in f32 and
applies the final Lorentz lift.

Device-side layout notes:
- Everything flows transposed ([feature, s]) so all matmuls contract on the
  partition axis without transposing x on device (host passes x^T).
- The 193-dim Lorentz q/k contraction is chunked [128 nope] + [64 rope + 1
  time]; the time rows sit at partition 64 of the 65-row "B" tiles.
- Rope dims are permuted even-pairs-first on the host so rotary is aligned
  [32, n] ops; weight columns are permuted to match.
- All matmul operands are bf16 (f32r at <256 moving columns runs at 1/4 PE
  rate); PSUM accumulation stays f32.
- Scalar engine uses ONLY the natural_log_exp activation table: every sqrt
  is computed as exp(0.5*ln(1+x)) so no ACT table reloads ever happen.
- Softmax max-pass skipped (scores <= 0 on the hyperboloid) and the softmax
  denominator cancels inside the Lorentz centroid normalization. V' carries
  its time coordinate in column 127 (wo rows are permuted on host to match).
- AV runs Vp-stationary: one N=512 matmul per j-tile accumulates aveT [d, q]
  in PSUM, so the centroid epilogue and the wo projection need no transposes.
  Scores for two j-tiles share one 2-bank PSUM tile; each exp call covers
  1024 columns, halving the scalar engine's per-call pipeline-fill cost.
- q/k time rows are batched: one-hot column-selector matmuls accumulate all
  (head, chunk) |.|^2 sums into an [8, 512] PSUM tile, so one ln/exp pair
  replaces sixteen 1-lane activation calls; squares run on the DVE (dual
  SBUF reads at 2x fp16 rate) instead of the scalar engine.
- The attention pair-loop is software-pipelined, and the previous group's wo
  matmuls are sprinkled between the score and (exp-gated) AV matmuls: the PE
  queue always holds independent work ahead of a semaphore wait, so it never
  micro-idles (micro-gaps hold the HAM clock gate at the 1.2 GHz K=4/8
  state; dense streams keep the 13/16 GPIO-limited 1.95 GHz).
- V' tiles are produced with PE transposes, drained on the scalar engine.
"""

import os
import sys
import types

import numpy as np
import ml_dtypes


def _ensure_axon_hooks():
    """Recreate the missing antenv.axon_hooks module so NTFF tracing works."""
    if "antenv.axon_hooks" in sys.modules:
        return
    try:
        import antenv
        from trn_agent_boot.trn_boot import _ntff_profile_via_ctypes

        hook = _ntff_profile_via_ctypes("/opt/axon/libaxon_pjrt.so")
        mod = types.ModuleType("antenv.axon_hooks")
        mod.get_axon_ntff_profile_hook = lambda: hook
        mod.set_axon_ntff_profile_hook = lambda h: None
        sys.modules["antenv.axon_hooks"] = mod
        antenv.axon_hooks = mod
    except Exception:
        pass


_ensure_axon_hooks()

import concourse.bacc as bacc
import concourse.bass as bass
import concourse.tile as tile
from concourse import mybir
import concourse.bass_utils as bass_utils
from concourse.bass_utils import run_bass_kernel_spmd
from concourse.masks import make_identity, make_upper_triangular

# zero-egress container: make the S3 artifact upload in the profile path a no-op
bass_utils.upload_artifacts = lambda tmpdir: tmpdir

F32 = mybir.dt.float32
BF16 = mybir.dt.float16  # 16-bit compute dtype (fp16: 10 mantissa bits)
FP8 = mybir.dt.float8e4  # e4m3, used for the latent gather payload
AF = mybir.ActivationFunctionType
AX = mybir.AxisListType
ALU = mybir.AluOpType

N_CORES = 8
P = 128
S = 2048          # sequence length
DIM = 2048        # model dim
NDC = DIM // P    # 16 contraction chunks over DIM
NQT = S // P      # 16 q/k tiles of 128
HPC = 2           # heads per core
NOPE = 128
RSP = 64          # rotary space dim
VSP = 127         # v space dim
KV_RANK = 512
EPS_RMS = 1e-6
QH = NOPE + RSP               # 192 q space rows per head
WQ_COLS = HPC * QH            # 384
WB_COLS = HPC * (NOPE + VSP)  # 510
WO_ROWS = HPC * P             # 256
OUT_COLS = DIM - 1            # 2047
NCH = 512                     # column chunk
NA = S // NCH                 # 4 chunks
SL = S // N_CORES             # 256
GR = KV_RANK + RSP + 1        # gathered rows: kvn + kpe + t_row


def _build_program(exp_scale: float, causal: bool):
    nc = bacc.Bacc("TRN2", target_bir_lowering=False, debug=False,
                   num_devices=N_CORES)

    xT_d = nc.dram_tensor("xT", [DIM, S], FP8, kind="ExternalInput")
    wq_d = nc.dram_tensor("wq", [DIM, WQ_COLS], BF16, kind="ExternalInput")
    wkva_d = nc.dram_tensor("wkva", [DIM, KV_RANK + RSP], BF16, kind="ExternalInput")
    wnormT_d = nc.dram_tensor("wnormT", [P, 4], F32, kind="ExternalInput")
    wkvb_d = nc.dram_tensor("wkvb", [KV_RANK + 1, WB_COLS], BF16, kind="ExternalInput")
    wo_d = nc.dram_tensor("wo", [WO_ROWS, OUT_COLS], BF16, kind="ExternalInput")
    cosT_d = nc.dram_tensor("cosT", [RSP, S], BF16, kind="ExternalInput")
    sinT_d = nc.dram_tensor("sinT", [RSP, S], BF16, kind="ExternalInput")
    l2_d = nc.dram_tensor("l2c", [P, 1], F32, kind="ExternalInput")
    lcc_d = nc.dram_tensor("lcc", [P, 128], BF16, kind="ExternalInput")
    out_d = nc.dram_tensor("out", [S, OUT_COLS], BF16, kind="ExternalOutput")
    xsl_d = nc.dram_tensor("xsl", [DIM, SL], BF16, kind="ExternalInput")
    cossl_d = nc.dram_tensor("cossl", [RSP, SL], BF16, kind="ExternalInput")
    sinsl_d = nc.dram_tensor("sinsl", [RSP, SL], BF16, kind="ExternalInput")
    gin = nc.dram_tensor("gin", [GR, SL], BF16)
    gout = nc.dram_tensor("gout", [N_CORES, GR, SL], BF16, addr_space="Shared")

    with tile.TileContext(nc) as tc:
        if os.environ.get("LMLA_NO_TABLE_PRELOAD") != "1":
            # Preload the combined ln+exp ACT table once; without this the
            # auto-placement pass alternates exp->table0 / ln->table5 loads
            # (1.28us each) all through the attention loop.
            nc.scalar.add_instruction(mybir.InstLoadActFuncSet(
                name=nc.get_next_instruction_name(), act_func_set_id=6,
                ins=[], outs=[]))
        const = tc.alloc_tile_pool(name="const", bufs=1)
        identity = const.tile([P, P], BF16)
        make_identity(nc, identity)
        diagmask = const.tile([P, P], BF16)
        make_upper_triangular(nc, diagmask, val=1.0, diag=True)
        wnormT = const.tile([P, 4], F32)
        nc.sync.dma_start(out=wnormT[:], in_=wnormT_d[:])
        Lt = const.tile([P, 4, 2], BF16)  # [ones | wnorm^2] per latent chunk
        for c in range(4):
            nc.vector.memset(Lt[:, c, 0:1], 1.0)
            nc.vector.tensor_mul(Lt[:, c, 1:2], wnormT[:, c:c + 1], wnormT[:, c:c + 1])
        ones_col = const.tile([P, 1], BF16)
        nc.vector.memset(ones_col[:], 1.0)
        ones_row = const.tile([1, P], F32)
        nc.vector.memset(ones_row[:], 1.0)
        ones_row_bf = const.tile([1, P], BF16)
        nc.vector.memset(ones_row_bf[:], 1.0)
        eps_b = const.tile([P, 1], F32)
        nc.vector.memset(eps_b[:], EPS_RMS)
        ln16_b = const.tile([P, 1], F32)
        nc.vector.memset(ln16_b[:], 2.772588722239781)

        # Long-lived tiles.
        big = tc.alloc_tile_pool(name="big", bufs=1)
        qsA = [big.tile([P, S], BF16, name=f"qsA_{h}", tag=f"qsA_{h}") for h in range(HPC)]
        qsB = [big.tile([RSP + 1, S], BF16, name=f"qsB_{h}", tag=f"qsB_{h}") for h in range(HPC)]
        kv = [big.tile([P, S], BF16, name=f"kv_{c}", tag=f"kv_{c}") for c in range(4)]
        kpe = big.tile([RSP, S], BF16, name="kpe", tag="kpe")
        ksB = [big.tile([RSP + 1, S], BF16, name=f"ksB_{h}", tag=f"ksB_{h}") for h in range(HPC)]
        Vp = [big.tile([P, NQT, P], BF16, name=f"Vp_{h}", tag=f"Vp_{h}") for h in range(HPC)]
        t_row_bf = big.tile([1, S], BF16, name="t_row_bf", tag="t_row_bf")

        # ------------- Slice phase: kv latent on this core's s-slice ---------
        p_wKV = tc.alloc_tile_pool(name="p_wKV", bufs=1)
        p_sl = tc.alloc_tile_pool(name="p_sl", bufs=1)
        p_pssl = tc.alloc_tile_pool(name="p_pssl", bufs=2, space="PSUM")
        wKV = []
        for dc in range(NDC):
            w = p_wKV.tile([P, KV_RANK + RSP], BF16, name=f"wKV_{dc}", tag=f"wKV_{dc}")
            nc.sync.dma_start(out=w[:], in_=wkva_d[dc * P:(dc + 1) * P, :])
            wKV.append(w)
        xsl_t = p_sl.tile([P, NDC, SL], BF16, name="xsl_t", tag="xsl_t")
        for dc in range(NDC):
            nc.sync.dma_start(out=xsl_t[:, dc, :],
                              in_=xsl_d[dc * P:(dc + 1) * P, :])
        cossl = p_sl.tile([RSP, SL], BF16, name="cossl", tag="cossl")
        sinsl = p_sl.tile([RSP, SL], BF16, name="sinsl", tag="sinsl")
        nc.sync.dma_start(out=cossl[:], in_=cossl_d[:])
        nc.sync.dma_start(out=sinsl[:], in_=sinsl_d[:])

        # phase-A weights prefetch during the slice compute (pure loads, no
        # waits, so they issue immediately on sync)
        p_wA = tc.alloc_tile_pool(name="p_wA", bufs=1)
        p_qsc = tc.alloc_tile_pool(name="p_qsc", bufs=1)
        cosT = p_qsc.tile([RSP, S], BF16, name="cosT", tag="cosT")
        sinT = p_qsc.tile([RSP, S], BF16, name="sinT", tag="sinT")
        nc.sync.dma_start(out=cosT[:], in_=cosT_d[:])
        nc.sync.dma_start(out=sinT[:], in_=sinT_d[:])
        wQ = []
        for dc in range(NDC):
            w = p_wA.tile([P, WQ_COLS], BF16, name=f"wQ_{dc}", tag=f"wQ_{dc}")
            nc.sync.dma_start(out=w[:], in_=wq_d[dc * P:(dc + 1) * P, :])
            wQ.append(w)

        kvsl = [p_sl.tile([P, SL], F32, name=f"kvsl_{c}", tag=f"kvsl_{c}")
                for c in range(4)]
        kpesl = p_sl.tile([RSP, SL], F32, name="kpesl", tag="kpesl")
        for c in range(4):
            ps = p_pssl.tile([P, SL], F32, name="psl", tag="psl", bufs=2)
            for dc in range(NDC):
                nc.tensor.matmul(ps[:], wKV[dc][:, c * P:(c + 1) * P],
                                 xsl_t[:, dc, :], start=(dc == 0), stop=(dc == NDC - 1))
            nc.vector.tensor_copy(kvsl[c][:], ps[:])
        ps = p_pssl.tile([P, SL], F32, name="psl", tag="psl", bufs=2)
        for dc in range(NDC):
            nc.tensor.matmul(ps[:RSP, :], wKV[dc][:, KV_RANK:],
                             xsl_t[:, dc, :], start=(dc == 0), stop=(dc == NDC - 1))
        nc.vector.tensor_copy(kpesl[:], ps[:RSP, :])

        # RMS stats on the slice
        ps_s = p_pssl.tile([1, SL], F32, name="ps_s", tag="ps_s", bufs=1)
        ps_w = p_pssl.tile([1, SL], F32, name="ps_w", tag="ps_w", bufs=1)
        for c in range(4):
            ksq = p_sl.tile([P, SL], BF16, name="ksq", tag="ksq", bufs=2)
            nc.scalar.square(ksq[:], kvsl[c][:])
            nc.tensor.matmul(ps_s[:], Lt[:, c, 0:1], ksq[:], start=(c == 0), stop=(c == 3))
            nc.tensor.matmul(ps_w[:], Lt[:, c, 1:2], ksq[:], start=(c == 0), stop=(c == 3))
        # inv_rms = exp(-0.5 * ln(mean_sq + eps)) ; single ACT table (ln/exp)
        ln_s = p_sl.tile([1, SL], F32, name="ln_s", tag="ln_s")
        nc.scalar.activation(ln_s[:], ps_s[:], AF.Ln, bias=eps_b[0:1, :],
                             scale=1.0 / KV_RANK)
        inv_rms = p_sl.tile([1, SL], F32, name="inv_rms", tag="inv_rms")
        nc.scalar.activation(inv_rms[:], ln_s[:], AF.Exp, scale=-0.5)
        tmp_r = p_sl.tile([1, SL], F32, name="tmp_r", tag="tmp_r")
        nc.vector.tensor_copy(tmp_r[:], ps_w[:])
        nc.vector.tensor_mul(tmp_r[:], tmp_r[:], inv_rms[:])
        nc.vector.tensor_mul(tmp_r[:], tmp_r[:], inv_rms[:])
        t_ln = p_sl.tile([1, SL], F32, name="t_ln", tag="t_ln")
        nc.scalar.activation(t_ln[:], tmp_r[:], AF.Ln, bias=1.0)
        t_st = p_sl.tile([1, SL], BF16, name="t_st", tag="t_st")
        nc.scalar.activation(t_st[:], t_ln[:], AF.Exp, scale=0.5)

        # broadcast inv_rms via outer product; fused scale -> bf16 stage
        rb = p_pssl.tile([P, SL], F32, name="rb", tag="rb", bufs=1)
        nc.tensor.matmul(rb[:], ones_row[:], inv_rms[:], start=True, stop=True)
        kvn_st = [p_sl.tile([P, SL], BF16, name=f"kvn_st_{c}", tag=f"kvn_st_{c}")
                  for c in range(4)]
        for c in range(4):
            nc.vector.scalar_tensor_tensor(
                kvn_st[c][:], kvsl[c][:], wnormT[:, c:c + 1], rb[:],
                op0=ALU.mult, op1=ALU.mult)

        # rotary on the k_pe slice
        rtl = p_sl.tile([RSP, SL], F32, name="rtl", tag="rtl")
        kpe_st = p_sl.tile([RSP, SL], BF16, name="kpe_st", tag="kpe_st")
        x0 = kpesl[0:32, :]
        x1 = kpesl[32:64, :]
        nc.vector.tensor_mul(rtl[32:64, :], x0, sinsl[0:32, :])
        nc.vector.tensor_mul(rtl[0:32, :], x1, sinsl[32:64, :])
        nc.vector.tensor_mul(x0, x0, cossl[0:32, :])
        nc.vector.tensor_mul(x1, x1, cossl[32:64, :])
        nc.vector.tensor_sub(kpe_st[0:32, :], x0, rtl[0:32, :])
        nc.vector.tensor_add(kpe_st[32:64, :], x1, rtl[32:64, :])

        # ship slice, gather full (single bf16 payload). The gin writes go on
        # the vector queue (their producers): on the in-order sync queue they
        # would block the phase-A weight/x DMA issues behind the slice tail.
        for c in range(4):
            nc.gpsimd.dma_start(out=gin[c * P:(c + 1) * P, :], in_=kvn_st[c][:])
        nc.gpsimd.dma_start(out=gin[KV_RANK:KV_RANK + RSP, :], in_=kpe_st[:])
        nc.gpsimd.dma_start(out=gin[KV_RANK + RSP:, :], in_=t_st[:])
        nc.gpsimd.collective_compute(
            "AllGather", ALU.bypass,
            replica_groups=[list(range(N_CORES))],
            ins=[gin[:]], outs=[gout[:]])
        # keep p_sl/p_wKV alive through phase A: recycling their SBUF for the
        # xt tiles makes the PE wait on the slice tail + gin DMA reads
        p_pssl.release()

        # --- Phase A: q projection over the full sequence --------------------
        # weight col layout (host): [qnope0 | qnope1 | qrope0(ev,od) | qrope1]
        # Per n-chunk: matmuls, drains to bf16, rotary (vector). q/k/v time
        # rows are all computed in phase B from the bf16 SBUF copies (DVE
        # squares + batched [8, 512] ln/exp).
        p_xs = tc.alloc_tile_pool(name="p_xs", bufs=1)
        p_psA = tc.alloc_tile_pool(name="p_psA", bufs=3, space="PSUM")

        for n in range(NA):
            n0 = n * NCH
            # x in fp8 (e4m3): noise enters the q side only (the slice/latent
            # path reads the separate bf16 xsl), halving the dominant 8.4MB
            # startup HBM stream so the gather triggers much earlier. Mixed
            # fp8xbf16 matmuls run at bf16 speed.
            xt = p_xs.tile([P, NDC, NCH], FP8, name="xt", tag="xt", bufs=2)
            src = xT_d[:, n0:n0 + NCH].rearrange("(dc p) s -> p dc s", p=P)
            for dc in range(NDC):
                nc.sync.dma_start(out=xt[:, dc, :], in_=src[:, dc, :])

            # rope chunk for both heads: rows [h0ev|h0od|h1ev|h1od]
            ps = p_psA.tile([P, NCH], F32, name="psa", tag="psa", bufs=3)
            for dc in range(NDC):
                nc.tensor.matmul(ps[:], wQ[dc][:, 2 * P:3 * P], xt[:, dc, :],
                                 start=(dc == 0), stop=(dc == NDC - 1))
            for h in range(HPC):
                nc.scalar.copy(qsB[h][0:RSP, n0:n0 + NCH], ps[h * RSP:(h + 1) * RSP, :])
            # rotary, in place on bf16 (2x DVE mode)
            rt = p_qsc.tile([RSP, NCH], BF16, name="rt", tag="rt", bufs=2)
            for h in range(HPC):
                gx0 = qsB[h][0:32, n0:n0 + NCH]
                gx1 = qsB[h][32:64, n0:n0 + NCH]
                nc.vector.tensor_mul(rt[32:64, :], gx0, sinT[0:32, n0:n0 + NCH])
                nc.vector.tensor_mul(rt[0:32, :], gx1, sinT[32:64, n0:n0 + NCH])
                nc.vector.tensor_mul(gx0, gx0, cosT[0:32, n0:n0 + NCH])
                nc.vector.tensor_mul(gx1, gx1, cosT[32:64, n0:n0 + NCH])
                nc.vector.tensor_sub(gx0, gx0, rt[0:32, :])
                nc.vector.tensor_add(gx1, gx1, rt[32:64, :])

            for h in range(HPC):
                ps = p_psA.tile([P, NCH], F32, name="psa", tag="psa", bufs=3)
                for dc in range(NDC):
                    nc.tensor.matmul(ps[:], wQ[dc][:, h * P:(h + 1) * P],
                                     xt[:, dc, :], start=(dc == 0), stop=(dc == NDC - 1))
                nc.vector.tensor_copy(qsA[h][:, n0:n0 + NCH], ps[:])
        p_psA.release()
        p_xs.release()
        p_qsc.release()
        p_wA.release()
        p_sl.release()
        p_wKV.release()

        # gather unpack. Emitted AFTER the phase-A pool releases and on the
        # (idle) gpsimd engine: the triggers wait on the collective, so on the
        # in-order sync engine they'd starve phase A's xt loads, and if they
        # precede the releases the release drain (queued behind them on
        # gpsimd) gates every post-release allocation.
        # kv unpack split by 512-col output chunk (k-slot pairs) so phase B's
        # first chunk matmuls fire as soon as their slice of the gather lands
        for nn in range(NA):
            for c in range(4):
                nc.gpsimd.dma_start(
                    out=kv[c][:, nn * NCH:(nn + 1) * NCH].rearrange(
                        "p (k s) -> p k s", k=2),
                    in_=gout[2 * nn:2 * nn + 2, c * P:(c + 1) * P, :].rearrange(
                        "k p s -> p k s"))
        # k_pe rows are shared by both heads: unpack straight into both ksB
        # tiles (and once more into kpe for the k-time squares)
        for h in range(HPC):
            nc.gpsimd.dma_start(
                out=ksB[h][0:RSP, :].rearrange("p (k s) -> p k s", k=N_CORES),
                in_=gout[:, KV_RANK:KV_RANK + RSP, :].rearrange("k p s -> p k s"))
        nc.gpsimd.dma_start(
            out=kpe[:].rearrange("p (k s) -> p k s", k=N_CORES),
            in_=gout[:, KV_RANK:KV_RANK + RSP, :].rearrange("k p s -> p k s"))
        nc.gpsimd.dma_start(
            out=t_row_bf[:].rearrange("p (k s) -> p k s", k=N_CORES),
            in_=gout[:, KV_RANK + RSP:, :].rearrange("k p s -> p k s"))

        # --- Phase B: kv_b projection + k/v time rows + V' assembly ----------
        big2 = tc.alloc_tile_pool(name="big2", bufs=1)
        p_wB = tc.alloc_tile_pool(name="p_wB", bufs=1)
        p_psB = tc.alloc_tile_pool(name="p_psB", bufs=3, space="PSUM")
        p_pkv = tc.alloc_tile_pool(name="p_pkv", bufs=2, space="PSUM")
        p_ptv = tc.alloc_tile_pool(name="p_ptv", bufs=2, space="PSUM")
        p_bsc = tc.alloc_tile_pool(name="p_bsc", bufs=1)
        wb_k = []
        for k in range(4):
            w = p_wB.tile([P, WB_COLS], BF16, name=f"wbk_{k}", tag=f"wbk_{k}")
            nc.sync.dma_start(out=w[:], in_=wkvb_d[k * P:(k + 1) * P, :])
            wb_k.append(w)
        wb_t = p_wB.tile([1, WB_COLS], BF16, name="wb_t", tag="wb_t")
        nc.sync.dma_start(out=wb_t[:], in_=wkvb_d[KV_RANK:KV_RANK + 1, :])

        ksA = [big2.tile([P, S], BF16, name=f"ksA_{h}", tag=f"ksA_{h}") for h in range(HPC)]
        vts = [big2.tile([P, S], BF16, name=f"vts_{h}", tag=f"vts_{h}") for h in range(HPC)]

        def kvb_mms(ps, col0, msize, n0):
            for k in range(4):
                nc.tensor.matmul(ps[:msize, :], wb_k[k][:, col0:col0 + msize],
                                 kv[k][:, n0:n0 + NCH], start=(k == 0), stop=False)
            nc.tensor.matmul(ps[:msize, :], wb_t[:, col0:col0 + msize],
                             t_row_bf[:, n0:n0 + NCH], start=False, stop=True)

        # batched time-row accumulators: rows r = h*4 + n of [8, NCH]; a
        # single ln/exp pair then covers all (h, n) at once. The one-hot
        # column selectors come from the host (lcc): zero columns write
        # zeros to the other rows, which is harmless under accumulation.
        lcc = p_wB.tile([P, 128], BF16, name="lcc", tag="lcc")
        nc.sync.dma_start(out=lcc[:], in_=lcc_d[:])
        qkall = p_pkv.tile([8, NCH], F32, name="qkall", tag="qkall", bufs=1)
        pkall = p_pkv.tile([8, NCH], F32, name="pkall", tag="pkall", bufs=1)

        for n in range(NA):
            n0 = n * NCH
            # DVE squares from the bf16 SBUF copies (dual SBUF reads, 2x rate)
            kpsq = p_bsc.tile([RSP, NCH], BF16, name="kpsq", tag="kpsq", bufs=2)
            nc.vector.tensor_mul(kpsq[:], kpe[:, n0:n0 + NCH],
                                 kpe[:, n0:n0 + NCH])
            qsq = p_bsc.tile([P, NCH], BF16, name="qsq", tag="qsq", bufs=2)
            for h in range(HPC):
                nc.vector.tensor_mul(qsq[h * RSP:(h + 1) * RSP, :],
                                     qsB[h][0:RSP, n0:n0 + NCH],
                                     qsB[h][0:RSP, n0:n0 + NCH])
            nc.tensor.matmul(qkall[:], lcc[:, 96 + 8 * n:96 + 8 * n + 8],
                             qsq[:], start=(n == 0), stop=False,
                             skip_group_check=True)
            for h in range(HPC):
                r = h * 4 + n
                qbsq = p_bsc.tile([P, NCH], BF16, name="qbsq", tag="qbsq", bufs=2)
                nc.vector.tensor_mul(qbsq[:], qsA[h][:, n0:n0 + NCH],
                                     qsA[h][:, n0:n0 + NCH])
                nc.tensor.matmul(qkall[:], lcc[:, 8 * r:8 * r + 8], qbsq[:],
                                 start=False, stop=(n == NA - 1 and h == HPC - 1),
                                 skip_group_check=True)
            for h in range(HPC):
                c0 = h * (NOPE + VSP)
                r = h * 4 + n
                # k_nope
                ps = p_psB.tile([P, NCH], F32, name="psb", tag="psb", bufs=3)
                kvb_mms(ps, c0, NOPE, n0)
                nc.vector.tensor_copy(ksA[h][:, n0:n0 + NCH], ps[:])
                bsq = p_bsc.tile([P, NCH], BF16, name="bsq", tag="bsq", bufs=2)
                nc.vector.tensor_mul(bsq[:], ksA[h][:, n0:n0 + NCH],
                                     ksA[h][:, n0:n0 + NCH])
                nc.tensor.matmul(pkall[:], lcc[:, 8 * r:8 * r + 8], bsq[:],
                                 start=(n == 0 and h == 0), stop=False,
                                 skip_group_check=True)
                if h == HPC - 1:
                    nc.tensor.matmul(pkall[:], lcc[0:RSP, 64 + 8 * n:64 + 8 * n + 8],
                                     kpsq[:],
                                     start=False, stop=(n == NA - 1),
                                     skip_group_check=True)
                # v (127 space rows; time goes in row 127 of vts)
                ps = p_psB.tile([P, NCH], F32, name="psb", tag="psb", bufs=3)
                kvb_mms(ps, c0 + NOPE, VSP, n0)
                nc.vector.tensor_copy(vts[h][0:VSP, n0:n0 + NCH], ps[:VSP, :])
                vsq = p_bsc.tile([VSP, NCH], BF16, name="vsq", tag="vsq", bufs=2)
                nc.vector.tensor_mul(vsq[:], vts[h][0:VSP, n0:n0 + NCH],
                                     vts[h][0:VSP, n0:n0 + NCH])
                pv = p_pkv.tile([1, NCH], F32, name="pv", tag="pv", bufs=1)
                nc.tensor.matmul(pv[:], ones_col[0:VSP, :], vsq[:],
                                 start=True, stop=True)
                vln = p_bsc.tile([1, NCH], F32, name="vln", tag="vln", bufs=2)
                nc.scalar.activation(vln[:], pv[:], AF.Ln, bias=1.0)
                # engines can't write a region based at partition 127; go via
                # a scratch row + SBUF->SBUF DMA
                vtr = p_bsc.tile([1, NCH], BF16, name="vtr", tag="vtr", bufs=2)
                nc.scalar.activation(vtr[:], vln[:], AF.Exp, scale=0.5)
                nc.sync.dma_start(out=vts[h][VSP:VSP + 1, n0:n0 + NCH],
                                  in_=vtr[:])
                # V' tiles for this chunk: PE transposes (DMA xbar transposes
                # get scheduled lazily, serialize against other DMAs, and
                # stall the attention loop's AV matmuls)
                for j in range(n * 4, n * 4 + 4):
                    tpv = p_ptv.tile([P, P], BF16, name="tpv", tag="tpv", bufs=2)
                    nc.tensor.transpose(tpv[:], vts[h][:, j * P:(j + 1) * P],
                                        identity[:])
                    nc.scalar.copy(Vp[h][:, j, :], tpv[:])
        # finalize the time rows: one ln/exp pair per quantity
        kt8 = p_bsc.tile([8, NCH], BF16, name="kt8", tag="kt8")
        kl8 = p_bsc.tile([8, NCH], F32, name="kl8", tag="kl8")
        nc.scalar.activation(kl8[:], pkall[:], AF.Ln, bias=1.0)
        nc.scalar.activation(kt8[:], kl8[:], AF.Exp, scale=0.5)
        qt8 = p_bsc.tile([8, NCH], BF16, name="qt8", tag="qt8")
        ql8 = p_bsc.tile([8, NCH], F32, name="ql8", tag="ql8")
        nc.scalar.activation(ql8[:], qkall[:], AF.Ln, bias=1.0)
        nc.scalar.activation(qt8[:], ql8[:], AF.Exp, scale=0.5)
        qt8n = p_bsc.tile([8, NCH], BF16, name="qt8n", tag="qt8n")
        nc.vector.tensor_scalar_mul(qt8n[:], qt8[:], -1.0)
        for h in range(HPC):
            for n in range(NA):
                r = h * 4 + n
                n0 = n * NCH
                nc.gpsimd.dma_start(out=ksB[h][RSP:RSP + 1, n0:n0 + NCH],
                                    in_=kt8[r:r + 1, :])
                nc.gpsimd.dma_start(out=qsB[h][RSP:RSP + 1, n0:n0 + NCH],
                                    in_=qt8n[r:r + 1, :])
        p_bsc.release()
        p_ptv.release()
        p_pkv.release()
        p_psB.release()
        p_wB.release()

        # ---------------- Phase C: attention ---------------------------------
        # scoresT layout [k, q]. AV runs Vp-stationary: one N=512 matmul per
        # j-tile accumulating aveT [d, q] in PSUM, so the epilogue and the wo
        # projection need no transposes at all. Scores for two j-tiles land in
        # one 2-bank PSUM tile so each exp call covers 1024 columns (the
        # scalar engine's per-call pipeline fill is ~290ns). The pair-loop is
        # software-pipelined two deep so the PE never waits on the exp.
        GQ = NCH // P
        NG = S // NCH
        p_ex = tc.alloc_tile_pool(name="p_ex", bufs=4)
        p_cw = tc.alloc_tile_pool(name="p_cw", bufs=2)
        p_wO = tc.alloc_tile_pool(name="p_wO", bufs=1)
        p_osb = tc.alloc_tile_pool(name="p_osb", bufs=4)
        p_ave = tc.alloc_tile_pool(name="p_ave", bufs=1, space="PSUM")
        p_scp = tc.alloc_tile_pool(name="p_scp", bufs=2, space="PSUM")
        p_pp = tc.alloc_tile_pool(name="p_pp", bufs=1, space="PSUM")
        p_psD = tc.alloc_tile_pool(name="p_psD", bufs=2, space="PSUM")

        wo_sb = []
        for h in range(HPC):
            w = p_wO.tile([P, OUT_COLS], BF16, name=f"wo_{h}", tag=f"wo_{h}")
            nc.sync.dma_start(out=w[:], in_=wo_d[h * P:(h + 1) * P, :])
            wo_sb.append(w)
        # Lsgn [P, 1] const: +1 at the time row (VSP), -1 elsewhere, so one
        # matmul against sq gives innr = t^2 - sum(space^2) directly.
        Lsgn = p_wO.tile([P, 1], F32, name="L2", tag="L2")
        nc.sync.dma_start(out=Lsgn[:], in_=l2_d[:])

        def sc_pair(g, h, jp, jmax):
            # scores for j-tiles jp, jp+1 into one [P, 2, NCH] fp16 (1-bank)
            # tile. Diagonal tiles compute full 512 q cols (masked cols are
            # real scores, zeroed in ex after the exp).
            c0 = g * NCH
            sc = p_scp.tile([P, 2, NCH], F32, name="sc", tag="sc", bufs=2)
            for dj in range(2):
                j = jp + dj
                nc.tensor.matmul(sc[:, dj, :], ksA[h][:, j * P:(j + 1) * P],
                                 qsA[h][:, c0:c0 + NCH], start=True, stop=False)
                nc.tensor.matmul(sc[:, dj, :], ksB[h][:, j * P:(j + 1) * P],
                                 qsB[h][:, c0:c0 + NCH], start=False, stop=True)
            return sc

        def exp_av(g, h, jp, ave, sc, jmax):
            ex = p_ex.tile([P, 2, NCH], BF16, name="ex", tag="ex", bufs=3)
            nc.scalar.activation(ex[:], sc[:], AF.Exp, scale=exp_scale)
            if causal:
                for dj in range(2):
                    j = jp + dj
                    d = j - g * GQ
                    if d >= 0:
                        if d > 0:
                            nc.vector.memset(ex[:, dj, 0:d * P], 0.0)
                        nc.vector.tensor_mul(ex[:, dj, d * P:(d + 1) * P],
                                             ex[:, dj, d * P:(d + 1) * P],
                                             diagmask[:])
            for dj in range(2):
                j = jp + dj
                nc.tensor.matmul(ave[:], Vp[h][:, j, :], ex[:, dj, :],
                                 start=(j == 0), stop=(j == jmax - 1))

        def epilogue(g, h, ave, cen2):
            # aveT [d, q]: innr per q col = t^2 - sum_d(space^2) > 0 (row VSP
            # is the time coord); one signed-sum matmul against Lsgn gives it
            # directly. rsv = 1/sqrt(innr) is broadcast to 128 partitions by
            # a 1-row outer-product matmul (engines reject stride-0 APs).
            sq = p_cw.tile([P, NCH], F32, name="sq", tag="sq", bufs=2)
            nc.scalar.square(sq[:], ave[:])
            # one PSUM bank reused: innr lands in row 0, then the broadcast
            # matmul overwrites the whole bank after the Ln has consumed it
            ppb = p_pp.tile([P, NCH], F32, name="ppb", tag="ppb", bufs=1)
            nc.tensor.matmul(ppb[0:1, :], Lsgn[:], sq[:], start=True, stop=True)
            lnr = p_cw.tile([1, NCH], F32, name="lnr", tag="lnr", bufs=2)
            nc.scalar.activation(lnr[:], ppb[0:1, :], AF.Ln)
            rsv = p_cw.tile([1, NCH], F32, name="rsv", tag="rsv", bufs=2)
            nc.scalar.activation(rsv[:], lnr[:], AF.Exp, scale=-0.5)
            nc.tensor.matmul(ppb[:], ones_row[:], rsv[:], start=True,
                             stop=True)
            rbs = p_cw.tile([P, NCH], BF16, name="rbs", tag="rbs", bufs=2)
            nc.vector.tensor_copy(rbs[:], ppb[:])
            nc.vector.tensor_mul(cen2[:, h, :], ave[:], rbs[:])

        # wo work is queued as (m, n) jobs and SPRINKLED between the score
        # matmuls and the exp-gated AV matmul: the PE queue then always holds
        # independent work ahead of the semaphore-waiting AV, so the engine
        # never micro-idles (micro-gaps hold the HAM clock gate at half rate).
        wo_jobs = []

        def wo_one(g, cen2, t, n, drain_scalar=False):
            m = g * GQ + t
            n0 = n * NCH
            nn = min(NCH, OUT_COLS - n0)
            ps = p_psD.tile([P, NCH], F32, name="psd", tag="psd", bufs=2)
            nc.tensor.matmul(ps[:, :nn], cen2[:, 0, t * P:(t + 1) * P],
                             wo_sb[0][:, n0:n0 + nn], start=True, stop=False)
            nc.tensor.matmul(ps[:, :nn], cen2[:, 1, t * P:(t + 1) * P],
                             wo_sb[1][:, n0:n0 + nn], start=False, stop=True)
            # drains stay off the scalar engine while the attention loop runs
            # (it gates the exp -> AV chain); the final flush alternates onto
            # the then-idle scalar engine. The 1/256 undoes the two x16
            # fp8 prescales (cen and wo).
            ot = p_osb.tile([P, NCH], BF16, name="ot", tag="ot", bufs=6)
            if drain_scalar:
                nc.scalar.copy(ot[:, :nn], ps[:, :nn])
            else:
                nc.vector.tensor_copy(ot[:, :nn], ps[:, :nn])
            nc.sync.dma_start(out=out_d[m * P:(m + 1) * P, n0:n0 + nn],
                              in_=ot[:, :nn])

        def wo_emit(k=1):
            for _ in range(k):
                if wo_jobs:
                    wo_jobs.pop(0)()

        prev_cen = None
        for g in range(NG):
            cen2 = p_cw.tile([P, 2, NCH], BF16, name="cen2", tag="cen2", bufs=2)
            for h in range(HPC):
                ave = p_ave.tile([P, NCH], F32, name="ave", tag="ave", bufs=1)
                jmax = (g * GQ + GQ) if causal else NQT
                pend = []
                for jp in range(0, jmax, 2):
                    pend.append((jp, sc_pair(g, h, jp, jmax)))
                    wo_emit()
                    if len(pend) > 1:
                        pj, psc = pend.pop(0)
                        exp_av(g, h, pj, ave, psc, jmax)
                for (pj, psc) in pend:
                    wo_emit()
                    exp_av(g, h, pj, ave, psc, jmax)
                epilogue(g, h, ave, cen2)
                wo_emit(2)
                if h == 0 and prev_cen is not None:
                    cp = prev_cen
                    wo_jobs.extend(
                        (lambda t=t, n=n, cp=cp, gg=g - 1: wo_one(gg, cp, t, n))
                        for t in range(GQ) for n in range(4))
            # drain leftovers before the next group's epilogue can wrap the
            # cen2 double-buffer ring
            wo_emit(len(wo_jobs))
            prev_cen = cen2
        for t in range(GQ):
            for n in range(4):
                wo_one(NG - 1, prev_cen, t, n, drain_scalar=(n % 2 == 1))

        p_psD.release()
        p_pp.release()
        p_scp.release()
        p_ave.release()
        p_osb.release()
        p_wO.release()
        p_cw.release()
        p_ex.release()

        big2.release()
        big.release()
        const.release()

    nc.compile()
    return nc


_CACHE = {}


def _get_program(exp_scale: float, causal: bool):
    key = (round(float(exp_scale), 12), causal)
    if key not in _CACHE:
        _CACHE[key] = _build_program(float(exp_scale), causal)
    return _CACHE[key]


def _rope_perm():
    """Even rope dims first, then odd (host-side column permutation)."""
    return np.concatenate([np.arange(0, RSP, 2), np.arange(1, RSP, 2)])


def kernel(x, start_pos, freqs_cos, freqs_sin, mask, wq_w, wkv_a_w, kv_norm_w,
           wkv_b_w, wo_w, softmax_scale, bias_p, _want_trace=False):
    x2 = np.ascontiguousarray(np.asarray(x, np.float32).reshape(S, DIM))
    xT = np.ascontiguousarray(x2.T)
    wq_w = np.asarray(wq_w, np.float32)
    wkv_a_w = np.asarray(wkv_a_w, np.float32)
    kv_norm_w = np.asarray(kv_norm_w, np.float32)
    wkv_b_w = np.asarray(wkv_b_w, np.float32)
    wo_w = np.asarray(wo_w, np.float32)
    cosT = np.asarray(freqs_cos, np.float32).T
    sinT = np.asarray(freqs_sin, np.float32).T
    cosT = np.ascontiguousarray(
        np.concatenate([cosT, cosT], axis=0).astype(np.float16))
    sinT = np.ascontiguousarray(
        np.concatenate([sinT, sinT], axis=0).astype(np.float16))

    mask = np.asarray(mask)
    causal = bool(np.array_equal(mask, np.triu(np.ones((S, S), bool), k=1)))
    if not causal:
        assert not mask.any(), "only causal or empty masks are supported"

    smax = float(np.asarray(softmax_scale).reshape(-1)[0])
    exp_scale = 2.0 / smax

    rp = _rope_perm()
    # wq per core-pair layout: [nope_h0 | nope_h1 | rope_h0(ev,od) | rope_h1(ev,od)]
    wq_r = wq_w.reshape(DIM, 16, QH)
    wq_nope = wq_r[:, :, :NOPE]                       # (DIM, 16, 128)
    wq_rope = wq_r[:, :, NOPE:][:, :, rp]             # (DIM, 16, 64) permuted
    wq_cores = []
    for c in range(N_CORES):
        h0, h1 = 2 * c, 2 * c + 1
        wq_cores.append(np.concatenate(
            [wq_nope[:, h0], wq_nope[:, h1], wq_rope[:, h0], wq_rope[:, h1]],
            axis=1))
    # wkva: [kv | rope-even | rope-odd]
    wkva_p = wkv_a_w.copy()
    wkva_p[:, KV_RANK:] = wkva_p[:, KV_RANK:][:, rp]
    # wkvb: kvn rows first, time row last
    wkvb_p = np.ascontiguousarray(np.concatenate([wkv_b_w[1:], wkv_b_w[:1]], axis=0))
    wnormT = np.ascontiguousarray(kv_norm_w.reshape(4, P).T)
    # wo rows per head: [v space (1..127), time (0)]
    wo_p = wo_w.reshape(16, P, OUT_COLS)
    wo_p = np.concatenate([wo_p[:, 1:, :], wo_p[:, 0:1, :]], axis=1)
    wo_p = wo_p.reshape(16 * P, OUT_COLS)

    nc = _get_program(exp_scale, causal)

    l2c = np.full((P, 1), -1.0, np.float32)
    l2c[VSP, 0] = 1.0

    # one-hot column selectors for the batched [8, 512] time-row reductions:
    # cols 0-63: slab r -> col r ones (full 128 rows), for bsq/qbsq (r=h*4+n)
    # cols 64-95: slab n -> cols {n, 4+n} ones on rows 0-63, for kpsq
    # cols 96-127: slab n -> col n on rows 0-63, col 4+n on rows 64-127 (qsq)
    lcc = np.zeros((P, 128), np.float16)
    for r in range(8):
        lcc[:, 8 * r + r] = 1.0
    for n4 in range(4):
        lcc[0:RSP, 64 + 8 * n4 + n4] = 1.0
        lcc[0:RSP, 64 + 8 * n4 + 4 + n4] = 1.0
        lcc[0:RSP, 96 + 8 * n4 + n4] = 1.0
        lcc[RSP:P, 96 + 8 * n4 + 4 + n4] = 1.0

    xT_bf = np.ascontiguousarray(xT.astype(np.float16))
    xT_f8 = np.ascontiguousarray(xT.astype(ml_dtypes.float8_e4m3))
    wkva_bf = np.ascontiguousarray(wkva_p.astype(np.float16))

    in_maps = []
    for c in range(N_CORES):
        in_maps.append({
            "xT": xT_f8,
            "wq": np.ascontiguousarray(wq_cores[c].astype(np.float16)),
            "wkva": wkva_bf,
            "wnormT": wnormT,
            "wkvb": np.ascontiguousarray(
                wkvb_p[:, c * WB_COLS:(c + 1) * WB_COLS].astype(np.float16)),
            "wo": np.ascontiguousarray(
                wo_p[c * WO_ROWS:(c + 1) * WO_ROWS, :].astype(np.float16)),
            "cosT": cosT,
            "sinT": sinT,
            "l2c": l2c,
            "lcc": lcc,
            "xsl": np.ascontiguousarray(xT_bf[:, c * SL:(c + 1) * SL]),
            "cossl": np.ascontiguousarray(cosT[:, c * SL:(c + 1) * SL]),
            "sinsl": np.ascontiguousarray(sinT[:, c * SL:(c + 1) * SL]),
        })

    res = run_bass_kernel_spmd(nc, in_maps, core_ids=list(range(N_CORES)),
                               trace=_want_trace)
    kernel.last_result = res

    total = res.results[0]["out"].astype(np.float32)
    for c in range(1, N_CORES):
        total = total + res.results[c]["out"].astype(np.float32)
    t = np.sqrt(np.sum(total * total, axis=-1, keepdims=True) + 1.0)
    out = np.concatenate([t, total], axis=-1)
    return out.reshape(1, S, DIM).astype(np.float32)



# revision 64
# speedup vs baseline: 1.1310x; 1.0758x over previous
"""Lorentz MLA attention kernel for Trainium2, sharded over 8 NeuronCores.

Sharding: tensor-parallel over the 16 attention heads (2 heads per core);
the kv_lora latent projection (wkv_a + RMS norm) is sequence-sharded and
AllGathered. The output projection wo is row-parallel: each core produces a
partial (2048, 2047) output in bf16; the host sums the 8 partials in f32 and
applies the final Lorentz lift.

Device-side layout notes:
- Everything flows transposed ([feature, s]) so all matmuls contract on the
  partition axis without transposing x on device (host passes x^T).
- The 193-dim Lorentz q/k contraction is chunked [128 nope] + [64 rope + 1
  time]; the time rows sit at partition 64 of the 65-row "B" tiles.
- Rope dims are permuted even-pairs-first on the host so rotary is aligned
  [32, n] ops; weight columns are permuted to match.
- All matmul operands are bf16 (f32r at <256 moving columns runs at 1/4 PE
  rate); PSUM accumulation stays f32.
- Scalar engine uses ONLY the natural_log_exp activation table: every sqrt
  is computed as exp(0.5*ln(1+x)) so no ACT table reloads ever happen.
- Softmax max-pass skipped (scores <= 0 on the hyperboloid) and the softmax
  denominator cancels inside the Lorentz centroid normalization. V' carries
  its time coordinate in column 127 (wo rows are permuted on host to match).
- AV runs Vp-stationary: one N=512 matmul per j-tile accumulates aveT [d, q]
  in PSUM, so the centroid epilogue and the wo projection need no transposes.
  Scores for two j-tiles share one 2-bank PSUM tile; each exp call covers
  1024 columns, halving the scalar engine's per-call pipeline-fill cost.
- q/k time rows are batched: one-hot column-selector matmuls accumulate all
  (head, chunk) |.|^2 sums into an [8, 512] PSUM tile, so one ln/exp pair
  replaces sixteen 1-lane activation calls; squares run on the DVE (dual
  SBUF reads at 2x fp16 rate) instead of the scalar engine.
- The attention pair-loop is software-pipelined, and the previous group's wo
  matmuls are sprinkled between the score and (exp-gated) AV matmuls: the PE
  queue always holds independent work ahead of a semaphore wait, so it never
  micro-idles (micro-gaps hold the HAM clock gate at the 1.2 GHz K=4/8
  state; dense streams keep the 13/16 GPIO-limited 1.95 GHz).
- V' tiles are produced with PE transposes, drained on the scalar engine.
"""

import os
import sys
import types

import numpy as np
import ml_dtypes


def _ensure_axon_hooks():
    """Recreate the missing antenv.axon_hooks module so NTFF tracing works."""
    if "antenv.axon_hooks" in sys.modules:
        return
    try:
        import antenv
        from trn_agent_boot.trn_boot import _ntff_profile_via_ctypes

        hook = _ntff_profile_via_ctypes("/opt/axon/libaxon_pjrt.so")
        mod = types.ModuleType("antenv.axon_hooks")
        mod.get_axon_ntff_profile_hook = lambda: hook
        mod.set_axon_ntff_profile_hook = lambda h: None
        sys.modules["antenv.axon_hooks"] = mod
        antenv.axon_hooks = mod
    except Exception:
        pass


_ensure_axon_hooks()

import concourse.bacc as bacc
import concourse.bass as bass
import concourse.tile as tile
from concourse import mybir
import concourse.bass_utils as bass_utils
from concourse.bass_utils import run_bass_kernel_spmd
from concourse.masks import make_identity, make_upper_triangular

# zero-egress container: make the S3 artifact upload in the profile path a no-op
bass_utils.upload_artifacts = lambda tmpdir: tmpdir

F32 = mybir.dt.float32
BF16 = mybir.dt.float16  # 16-bit compute dtype (fp16: 10 mantissa bits)
FP8 = mybir.dt.float8e4  # e4m3, used for the latent gather payload
AF = mybir.ActivationFunctionType
AX = mybir.AxisListType
ALU = mybir.AluOpType

N_CORES = 8
P = 128
S = 2048          # sequence length
DIM = 2048        # model dim
NDC = DIM // P    # 16 contraction chunks over DIM
NQT = S // P      # 16 q/k tiles of 128
HPC = 2           # heads per core
NOPE = 128
RSP = 64          # rotary space dim
VSP = 127         # v space dim
KV_RANK = 512
EPS_RMS = 1e-6
QH = NOPE + RSP               # 192 q space rows per head
WQ_COLS = HPC * QH            # 384
WB_COLS = HPC * (NOPE + VSP)  # 510
WO_ROWS = HPC * P             # 256
OUT_COLS = DIM - 1            # 2047
NCH = 512                     # column chunk
NA = S // NCH                 # 4 chunks
SL = S // N_CORES             # 256
GR = KV_RANK + RSP + 1        # gathered rows: kvn + kpe + t_row


def _build_program(exp_scale: float, causal: bool):
    nc = bacc.Bacc("TRN2", target_bir_lowering=False, debug=False,
                   num_devices=N_CORES)

    xT_d = nc.dram_tensor("xT", [DIM, S], FP8, kind="ExternalInput")
    wq_d = nc.dram_tensor("wq", [DIM, WQ_COLS], BF16, kind="ExternalInput")
    wkva_d = nc.dram_tensor("wkva", [DIM, KV_RANK + RSP], BF16, kind="ExternalInput")
    wnormT_d = nc.dram_tensor("wnormT", [P, 4], F32, kind="ExternalInput")
    wkvb_d = nc.dram_tensor("wkvb", [KV_RANK + 1, WB_COLS], BF16, kind="ExternalInput")
    wo_d = nc.dram_tensor("wo", [WO_ROWS, OUT_COLS], BF16, kind="ExternalInput")
    cosT_d = nc.dram_tensor("cosT", [RSP, S], BF16, kind="ExternalInput")
    sinT_d = nc.dram_tensor("sinT", [RSP, S], BF16, kind="ExternalInput")
    l2_d = nc.dram_tensor("l2c", [P, 1], F32, kind="ExternalInput")
    lcc_d = nc.dram_tensor("lcc", [P, 128], BF16, kind="ExternalInput")
    out_d = nc.dram_tensor("out", [S, OUT_COLS], BF16, kind="ExternalOutput")
    xsl_d = nc.dram_tensor("xsl", [DIM, SL], BF16, kind="ExternalInput")
    cossl_d = nc.dram_tensor("cossl", [RSP, SL], BF16, kind="ExternalInput")
    sinsl_d = nc.dram_tensor("sinsl", [RSP, SL], BF16, kind="ExternalInput")
    gin = nc.dram_tensor("gin", [GR, SL], BF16)
    gout = nc.dram_tensor("gout", [N_CORES, GR, SL], BF16, addr_space="Shared")

    with tile.TileContext(nc) as tc:
        if os.environ.get("LMLA_NO_TABLE_PRELOAD") != "1":
            # Preload the combined ln+exp ACT table once; without this the
            # auto-placement pass alternates exp->table0 / ln->table5 loads
            # (1.28us each) all through the attention loop.
            nc.scalar.add_instruction(mybir.InstLoadActFuncSet(
                name=nc.get_next_instruction_name(), act_func_set_id=6,
                ins=[], outs=[]))
        const = tc.alloc_tile_pool(name="const", bufs=1)
        identity = const.tile([P, P], BF16)
        make_identity(nc, identity)
        diagmask = const.tile([P, P], BF16)
        make_upper_triangular(nc, diagmask, val=1.0, diag=True)
        wnormT = const.tile([P, 4], F32)
        nc.sync.dma_start(out=wnormT[:], in_=wnormT_d[:])
        Lt = const.tile([P, 4, 2], BF16)  # [ones | wnorm^2] per latent chunk
        for c in range(4):
            nc.vector.memset(Lt[:, c, 0:1], 1.0)
            nc.vector.tensor_mul(Lt[:, c, 1:2], wnormT[:, c:c + 1], wnormT[:, c:c + 1])
        ones_col = const.tile([P, 1], BF16)
        nc.vector.memset(ones_col[:], 1.0)
        ones_row = const.tile([1, P], F32)
        nc.vector.memset(ones_row[:], 1.0)
        ones_row_bf = const.tile([1, P], BF16)
        nc.vector.memset(ones_row_bf[:], 1.0)
        eps_b = const.tile([P, 1], F32)
        nc.vector.memset(eps_b[:], EPS_RMS)
        ln16_b = const.tile([P, 1], F32)
        nc.vector.memset(ln16_b[:], 2.772588722239781)

        # Long-lived tiles.
        big = tc.alloc_tile_pool(name="big", bufs=1)
        qsA = [big.tile([P, S], BF16, name=f"qsA_{h}", tag=f"qsA_{h}") for h in range(HPC)]
        qsB = [big.tile([RSP + 1, S], BF16, name=f"qsB_{h}", tag=f"qsB_{h}") for h in range(HPC)]
        kv = [big.tile([P, S], BF16, name=f"kv_{c}", tag=f"kv_{c}") for c in range(4)]
        kpe = big.tile([RSP, S], BF16, name="kpe", tag="kpe")
        ksB = [big.tile([RSP + 1, S], BF16, name=f"ksB_{h}", tag=f"ksB_{h}") for h in range(HPC)]
        Vp = [big.tile([P, NQT, P], BF16, name=f"Vp_{h}", tag=f"Vp_{h}") for h in range(HPC)]
        t_row_bf = big.tile([1, S], BF16, name="t_row_bf", tag="t_row_bf")

        # ------------- Slice phase: kv latent on this core's s-slice ---------
        p_wKV = tc.alloc_tile_pool(name="p_wKV", bufs=1)
        p_sl = tc.alloc_tile_pool(name="p_sl", bufs=1)
        p_pssl = tc.alloc_tile_pool(name="p_pssl", bufs=2, space="PSUM")
        wKV = []
        for dc in range(NDC):
            w = p_wKV.tile([P, KV_RANK + RSP], BF16, name=f"wKV_{dc}", tag=f"wKV_{dc}")
            nc.sync.dma_start(out=w[:], in_=wkva_d[dc * P:(dc + 1) * P, :])
            wKV.append(w)
        xsl_t = p_sl.tile([P, NDC, SL], BF16, name="xsl_t", tag="xsl_t")
        for dc in range(NDC):
            nc.sync.dma_start(out=xsl_t[:, dc, :],
                              in_=xsl_d[dc * P:(dc + 1) * P, :])
        cossl = p_sl.tile([RSP, SL], BF16, name="cossl", tag="cossl")
        sinsl = p_sl.tile([RSP, SL], BF16, name="sinsl", tag="sinsl")
        nc.sync.dma_start(out=cossl[:], in_=cossl_d[:])
        nc.sync.dma_start(out=sinsl[:], in_=sinsl_d[:])

        # phase-A weights prefetch during the slice compute (pure loads, no
        # waits, so they issue immediately on sync)
        p_wA = tc.alloc_tile_pool(name="p_wA", bufs=1)
        p_qsc = tc.alloc_tile_pool(name="p_qsc", bufs=1)
        cosT = p_qsc.tile([RSP, S], BF16, name="cosT", tag="cosT")
        sinT = p_qsc.tile([RSP, S], BF16, name="sinT", tag="sinT")
        nc.sync.dma_start(out=cosT[:], in_=cosT_d[:])
        nc.sync.dma_start(out=sinT[:], in_=sinT_d[:])
        wQ = []
        for dc in range(NDC):
            w = p_wA.tile([P, WQ_COLS], BF16, name=f"wQ_{dc}", tag=f"wQ_{dc}")
            nc.sync.dma_start(out=w[:], in_=wq_d[dc * P:(dc + 1) * P, :])
            wQ.append(w)

        kvsl = [p_sl.tile([P, SL], F32, name=f"kvsl_{c}", tag=f"kvsl_{c}")
                for c in range(4)]
        kpesl = p_sl.tile([RSP, SL], F32, name="kpesl", tag="kpesl")
        for c in range(4):
            ps = p_pssl.tile([P, SL], F32, name="psl", tag="psl", bufs=2)
            for dc in range(NDC):
                nc.tensor.matmul(ps[:], wKV[dc][:, c * P:(c + 1) * P],
                                 xsl_t[:, dc, :], start=(dc == 0), stop=(dc == NDC - 1))
            nc.vector.tensor_copy(kvsl[c][:], ps[:])
        ps = p_pssl.tile([P, SL], F32, name="psl", tag="psl", bufs=2)
        for dc in range(NDC):
            nc.tensor.matmul(ps[:RSP, :], wKV[dc][:, KV_RANK:],
                             xsl_t[:, dc, :], start=(dc == 0), stop=(dc == NDC - 1))
        nc.vector.tensor_copy(kpesl[:], ps[:RSP, :])

        # RMS stats on the slice
        ps_s = p_pssl.tile([1, SL], F32, name="ps_s", tag="ps_s", bufs=1)
        ps_w = p_pssl.tile([1, SL], F32, name="ps_w", tag="ps_w", bufs=1)
        for c in range(4):
            ksq = p_sl.tile([P, SL], BF16, name="ksq", tag="ksq", bufs=2)
            nc.scalar.square(ksq[:], kvsl[c][:])
            nc.tensor.matmul(ps_s[:], Lt[:, c, 0:1], ksq[:], start=(c == 0), stop=(c == 3))
            nc.tensor.matmul(ps_w[:], Lt[:, c, 1:2], ksq[:], start=(c == 0), stop=(c == 3))
        # inv_rms = exp(-0.5 * ln(mean_sq + eps)) ; single ACT table (ln/exp)
        ln_s = p_sl.tile([1, SL], F32, name="ln_s", tag="ln_s")
        nc.scalar.activation(ln_s[:], ps_s[:], AF.Ln, bias=eps_b[0:1, :],
                             scale=1.0 / KV_RANK)
        inv_rms = p_sl.tile([1, SL], F32, name="inv_rms", tag="inv_rms")
        nc.scalar.activation(inv_rms[:], ln_s[:], AF.Exp, scale=-0.5)
        tmp_r = p_sl.tile([1, SL], F32, name="tmp_r", tag="tmp_r")
        nc.vector.tensor_copy(tmp_r[:], ps_w[:])
        nc.vector.tensor_mul(tmp_r[:], tmp_r[:], inv_rms[:])
        nc.vector.tensor_mul(tmp_r[:], tmp_r[:], inv_rms[:])
        t_ln = p_sl.tile([1, SL], F32, name="t_ln", tag="t_ln")
        nc.scalar.activation(t_ln[:], tmp_r[:], AF.Ln, bias=1.0)
        t_st = p_sl.tile([1, SL], BF16, name="t_st", tag="t_st")
        nc.scalar.activation(t_st[:], t_ln[:], AF.Exp, scale=0.5)

        # broadcast inv_rms via outer product; fused scale -> bf16 stage
        rb = p_pssl.tile([P, SL], F32, name="rb", tag="rb", bufs=1)
        nc.tensor.matmul(rb[:], ones_row[:], inv_rms[:], start=True, stop=True)
        kvn_st = [p_sl.tile([P, SL], BF16, name=f"kvn_st_{c}", tag=f"kvn_st_{c}")
                  for c in range(4)]
        for c in range(4):
            nc.vector.scalar_tensor_tensor(
                kvn_st[c][:], kvsl[c][:], wnormT[:, c:c + 1], rb[:],
                op0=ALU.mult, op1=ALU.mult)

        # rotary on the k_pe slice
        rtl = p_sl.tile([RSP, SL], F32, name="rtl", tag="rtl")
        kpe_st = p_sl.tile([RSP, SL], BF16, name="kpe_st", tag="kpe_st")
        x0 = kpesl[0:32, :]
        x1 = kpesl[32:64, :]
        nc.vector.tensor_mul(rtl[32:64, :], x0, sinsl[0:32, :])
        nc.vector.tensor_mul(rtl[0:32, :], x1, sinsl[32:64, :])
        nc.vector.tensor_mul(x0, x0, cossl[0:32, :])
        nc.vector.tensor_mul(x1, x1, cossl[32:64, :])
        nc.vector.tensor_sub(kpe_st[0:32, :], x0, rtl[0:32, :])
        nc.vector.tensor_add(kpe_st[32:64, :], x1, rtl[32:64, :])

        # ship slice, gather full (single bf16 payload). The gin writes go on
        # the vector queue (their producers): on the in-order sync queue they
        # would block the phase-A weight/x DMA issues behind the slice tail.
        for c in range(4):
            nc.gpsimd.dma_start(out=gin[c * P:(c + 1) * P, :], in_=kvn_st[c][:])
        nc.gpsimd.dma_start(out=gin[KV_RANK:KV_RANK + RSP, :], in_=kpe_st[:])
        nc.gpsimd.dma_start(out=gin[KV_RANK + RSP:, :], in_=t_st[:])
        nc.gpsimd.collective_compute(
            "AllGather", ALU.bypass,
            replica_groups=[list(range(N_CORES))],
            ins=[gin[:]], outs=[gout[:]])
        # keep p_sl/p_wKV alive through phase A: recycling their SBUF for the
        # xt tiles makes the PE wait on the slice tail + gin DMA reads
        p_pssl.release()

        # --- Phase A: q projection over the full sequence --------------------
        # weight col layout (host): [qnope0 | qnope1 | qrope0(ev,od) | qrope1]
        # Per n-chunk: matmuls, drains to bf16, rotary (vector). q/k/v time
        # rows are all computed in phase B from the bf16 SBUF copies (DVE
        # squares + batched [8, 512] ln/exp).
        p_xs = tc.alloc_tile_pool(name="p_xs", bufs=1)
        p_psA = tc.alloc_tile_pool(name="p_psA", bufs=3, space="PSUM")

        for n in range(NA):
            n0 = n * NCH
            # x in fp8 (e4m3): noise enters the q side only (the slice/latent
            # path reads the separate bf16 xsl), halving the dominant 8.4MB
            # startup HBM stream so the gather triggers much earlier. Mixed
            # fp8xbf16 matmuls run at bf16 speed.
            xt = p_xs.tile([P, NDC, NCH], FP8, name="xt", tag="xt", bufs=2)
            src = xT_d[:, n0:n0 + NCH].rearrange("(dc p) s -> p dc s", p=P)
            for dc in range(NDC):
                nc.sync.dma_start(out=xt[:, dc, :], in_=src[:, dc, :])

            # rope chunk for both heads: rows [h0ev|h0od|h1ev|h1od]
            ps = p_psA.tile([P, NCH], F32, name="psa", tag="psa", bufs=3)
            for dc in range(NDC):
                nc.tensor.matmul(ps[:], wQ[dc][:, 2 * P:3 * P], xt[:, dc, :],
                                 start=(dc == 0), stop=(dc == NDC - 1))
            for h in range(HPC):
                nc.scalar.copy(qsB[h][0:RSP, n0:n0 + NCH], ps[h * RSP:(h + 1) * RSP, :])
            # rotary, in place on bf16 (2x DVE mode)
            rt = p_qsc.tile([RSP, NCH], BF16, name="rt", tag="rt", bufs=2)
            for h in range(HPC):
                gx0 = qsB[h][0:32, n0:n0 + NCH]
                gx1 = qsB[h][32:64, n0:n0 + NCH]
                nc.vector.tensor_mul(rt[32:64, :], gx0, sinT[0:32, n0:n0 + NCH])
                nc.vector.tensor_mul(rt[0:32, :], gx1, sinT[32:64, n0:n0 + NCH])
                nc.vector.tensor_mul(gx0, gx0, cosT[0:32, n0:n0 + NCH])
                nc.vector.tensor_mul(gx1, gx1, cosT[32:64, n0:n0 + NCH])
                nc.vector.tensor_sub(gx0, gx0, rt[0:32, :])
                nc.vector.tensor_add(gx1, gx1, rt[32:64, :])

            for h in range(HPC):
                ps = p_psA.tile([P, NCH], F32, name="psa", tag="psa", bufs=3)
                for dc in range(NDC):
                    nc.tensor.matmul(ps[:], wQ[dc][:, h * P:(h + 1) * P],
                                     xt[:, dc, :], start=(dc == 0), stop=(dc == NDC - 1))
                nc.vector.tensor_copy(qsA[h][:, n0:n0 + NCH], ps[:])
        p_psA.release()
        p_xs.release()
        p_qsc.release()
        p_wA.release()
        p_sl.release()
        p_wKV.release()

        # gather unpack. Emitted AFTER the phase-A pool releases and on the
        # (idle) gpsimd engine: the triggers wait on the collective, so on the
        # in-order sync engine they'd starve phase A's xt loads, and if they
        # precede the releases the release drain (queued behind them on
        # gpsimd) gates every post-release allocation.
        # Small tiles first: every phase-B accumulation group ends with the
        # wb_t matmul (needs t_row) and the squares need kpe, so these 2KB /
        # 0.5MB unpacks must not queue behind the 1MB kv unpack.
        nc.gpsimd.dma_start(
            out=t_row_bf[:].rearrange("p (k s) -> p k s", k=N_CORES),
            in_=gout[:, KV_RANK + RSP:, :].rearrange("k p s -> p k s"))
        nc.gpsimd.dma_start(
            out=kpe[:].rearrange("p (k s) -> p k s", k=N_CORES),
            in_=gout[:, KV_RANK:KV_RANK + RSP, :].rearrange("k p s -> p k s"))
        # kv unpack split by 512-col output chunk (k-slot pairs) so phase B's
        # first chunk matmuls fire as soon as their slice of the gather lands
        for nn in range(NA):
            for c in range(4):
                nc.gpsimd.dma_start(
                    out=kv[c][:, nn * NCH:(nn + 1) * NCH].rearrange(
                        "p (k s) -> p k s", k=2),
                    in_=gout[2 * nn:2 * nn + 2, c * P:(c + 1) * P, :].rearrange(
                        "k p s -> p k s"))
        # k_pe rows are shared by both heads: unpack straight into both ksB
        # tiles (scores-B reads them only in phase C)
        for h in range(HPC):
            nc.gpsimd.dma_start(
                out=ksB[h][0:RSP, :].rearrange("p (k s) -> p k s", k=N_CORES),
                in_=gout[:, KV_RANK:KV_RANK + RSP, :].rearrange("k p s -> p k s"))

        # --- Phase B: kv_b projection + k/v time rows + V' assembly ----------
        big2 = tc.alloc_tile_pool(name="big2", bufs=1)
        p_wB = tc.alloc_tile_pool(name="p_wB", bufs=1)
        p_psB = tc.alloc_tile_pool(name="p_psB", bufs=3, space="PSUM")
        p_pkv = tc.alloc_tile_pool(name="p_pkv", bufs=2, space="PSUM")
        p_ptv = tc.alloc_tile_pool(name="p_ptv", bufs=2, space="PSUM")
        p_bsc = tc.alloc_tile_pool(name="p_bsc", bufs=1)
        wb_k = []
        for k in range(4):
            w = p_wB.tile([P, WB_COLS], BF16, name=f"wbk_{k}", tag=f"wbk_{k}")
            nc.sync.dma_start(out=w[:], in_=wkvb_d[k * P:(k + 1) * P, :])
            wb_k.append(w)
        wb_t = p_wB.tile([1, WB_COLS], BF16, name="wb_t", tag="wb_t")
        nc.sync.dma_start(out=wb_t[:], in_=wkvb_d[KV_RANK:KV_RANK + 1, :])

        ksA = [big2.tile([P, S], BF16, name=f"ksA_{h}", tag=f"ksA_{h}") for h in range(HPC)]
        vts = [big2.tile([P, S], BF16, name=f"vts_{h}", tag=f"vts_{h}") for h in range(HPC)]

        def kvb_mms(ps, col0, msize, n0):
            for k in range(4):
                nc.tensor.matmul(ps[:msize, :], wb_k[k][:, col0:col0 + msize],
                                 kv[k][:, n0:n0 + NCH], start=(k == 0), stop=False)
            nc.tensor.matmul(ps[:msize, :], wb_t[:, col0:col0 + msize],
                             t_row_bf[:, n0:n0 + NCH], start=False, stop=True)

        # batched time-row accumulators: rows r = h*4 + n of [8, NCH]; a
        # single ln/exp pair then covers all (h, n) at once. The one-hot
        # column selectors come from the host (lcc): zero columns write
        # zeros to the other rows, which is harmless under accumulation.
        lcc = p_wB.tile([P, 128], BF16, name="lcc", tag="lcc")
        nc.sync.dma_start(out=lcc[:], in_=lcc_d[:])
        qkall = p_pkv.tile([8, NCH], F32, name="qkall", tag="qkall", bufs=1)
        pkall = p_pkv.tile([8, NCH], F32, name="pkall", tag="pkall", bufs=1)

        for n in range(NA):
            n0 = n * NCH
            # DVE squares from the bf16 SBUF copies (dual SBUF reads, 2x rate)
            kpsq = p_bsc.tile([RSP, NCH], BF16, name="kpsq", tag="kpsq", bufs=2)
            nc.vector.tensor_mul(kpsq[:], kpe[:, n0:n0 + NCH],
                                 kpe[:, n0:n0 + NCH])
            qsq = p_bsc.tile([P, NCH], BF16, name="qsq", tag="qsq", bufs=2)
            for h in range(HPC):
                nc.vector.tensor_mul(qsq[h * RSP:(h + 1) * RSP, :],
                                     qsB[h][0:RSP, n0:n0 + NCH],
                                     qsB[h][0:RSP, n0:n0 + NCH])
            nc.tensor.matmul(qkall[:], lcc[:, 96 + 8 * n:96 + 8 * n + 8],
                             qsq[:], start=(n == 0), stop=False,
                             skip_group_check=True)
            for h in range(HPC):
                r = h * 4 + n
                qbsq = p_bsc.tile([P, NCH], BF16, name="qbsq", tag="qbsq", bufs=2)
                nc.vector.tensor_mul(qbsq[:], qsA[h][:, n0:n0 + NCH],
                                     qsA[h][:, n0:n0 + NCH])
                nc.tensor.matmul(qkall[:], lcc[:, 8 * r:8 * r + 8], qbsq[:],
                                 start=False, stop=(n == NA - 1 and h == HPC - 1),
                                 skip_group_check=True)
            for h in range(HPC):
                c0 = h * (NOPE + VSP)
                r = h * 4 + n
                # k_nope
                ps = p_psB.tile([P, NCH], F32, name="psb", tag="psb", bufs=3)
                kvb_mms(ps, c0, NOPE, n0)
                nc.vector.tensor_copy(ksA[h][:, n0:n0 + NCH], ps[:])
                bsq = p_bsc.tile([P, NCH], BF16, name="bsq", tag="bsq", bufs=2)
                nc.vector.tensor_mul(bsq[:], ksA[h][:, n0:n0 + NCH],
                                     ksA[h][:, n0:n0 + NCH])
                nc.tensor.matmul(pkall[:], lcc[:, 8 * r:8 * r + 8], bsq[:],
                                 start=(n == 0 and h == 0), stop=False,
                                 skip_group_check=True)
                if h == HPC - 1:
                    nc.tensor.matmul(pkall[:], lcc[0:RSP, 64 + 8 * n:64 + 8 * n + 8],
                                     kpsq[:],
                                     start=False, stop=(n == NA - 1),
                                     skip_group_check=True)
                # v (127 space rows; time goes in row 127 of vts)
                ps = p_psB.tile([P, NCH], F32, name="psb", tag="psb", bufs=3)
                kvb_mms(ps, c0 + NOPE, VSP, n0)
                nc.vector.tensor_copy(vts[h][0:VSP, n0:n0 + NCH], ps[:VSP, :])
                vsq = p_bsc.tile([VSP, NCH], BF16, name="vsq", tag="vsq", bufs=2)
                nc.vector.tensor_mul(vsq[:], vts[h][0:VSP, n0:n0 + NCH],
                                     vts[h][0:VSP, n0:n0 + NCH])
                pv = p_pkv.tile([1, NCH], F32, name="pv", tag="pv", bufs=1)
                nc.tensor.matmul(pv[:], ones_col[0:VSP, :], vsq[:],
                                 start=True, stop=True)
                vln = p_bsc.tile([1, NCH], F32, name="vln", tag="vln", bufs=2)
                nc.scalar.activation(vln[:], pv[:], AF.Ln, bias=1.0)
                # engines can't write a region based at partition 127; go via
                # a scratch row + SBUF->SBUF DMA
                vtr = p_bsc.tile([1, NCH], BF16, name="vtr", tag="vtr", bufs=2)
                nc.scalar.activation(vtr[:], vln[:], AF.Exp, scale=0.5)
                nc.sync.dma_start(out=vts[h][VSP:VSP + 1, n0:n0 + NCH],
                                  in_=vtr[:])
                # V' tiles for this chunk: PE transposes (DMA xbar transposes
                # get scheduled lazily, serialize against other DMAs, and
                # stall the attention loop's AV matmuls)
                for j in range(n * 4, n * 4 + 4):
                    tpv = p_ptv.tile([P, P], BF16, name="tpv", tag="tpv", bufs=2)
                    nc.tensor.transpose(tpv[:], vts[h][:, j * P:(j + 1) * P],
                                        identity[:])
                    nc.scalar.copy(Vp[h][:, j, :], tpv[:])
        # finalize the time rows: one ln/exp pair per quantity
        kt8 = p_bsc.tile([8, NCH], BF16, name="kt8", tag="kt8")
        kl8 = p_bsc.tile([8, NCH], F32, name="kl8", tag="kl8")
        nc.scalar.activation(kl8[:], pkall[:], AF.Ln, bias=1.0)
        nc.scalar.activation(kt8[:], kl8[:], AF.Exp, scale=0.5)
        qt8 = p_bsc.tile([8, NCH], BF16, name="qt8", tag="qt8")
        ql8 = p_bsc.tile([8, NCH], F32, name="ql8", tag="ql8")
        nc.scalar.activation(ql8[:], qkall[:], AF.Ln, bias=1.0)
        nc.scalar.activation(qt8[:], ql8[:], AF.Exp, scale=0.5)
        qt8n = p_bsc.tile([8, NCH], BF16, name="qt8n", tag="qt8n")
        nc.vector.tensor_scalar_mul(qt8n[:], qt8[:], -1.0)
        for h in range(HPC):
            for n in range(NA):
                r = h * 4 + n
                n0 = n * NCH
                nc.gpsimd.dma_start(out=ksB[h][RSP:RSP + 1, n0:n0 + NCH],
                                    in_=kt8[r:r + 1, :])
                nc.gpsimd.dma_start(out=qsB[h][RSP:RSP + 1, n0:n0 + NCH],
                                    in_=qt8n[r:r + 1, :])
        p_bsc.release()
        p_ptv.release()
        p_pkv.release()
        p_psB.release()
        p_wB.release()

        # ---------------- Phase C: attention ---------------------------------
        # scoresT layout [k, q]. AV runs Vp-stationary: one N=512 matmul per
        # j-tile accumulating aveT [d, q] in PSUM, so the epilogue and the wo
        # projection need no transposes at all. Scores for two j-tiles land in
        # one 2-bank PSUM tile so each exp call covers 1024 columns (the
        # scalar engine's per-call pipeline fill is ~290ns). The pair-loop is
        # software-pipelined two deep so the PE never waits on the exp.
        GQ = NCH // P
        NG = S // NCH
        p_ex = tc.alloc_tile_pool(name="p_ex", bufs=4)
        p_cw = tc.alloc_tile_pool(name="p_cw", bufs=2)
        p_wO = tc.alloc_tile_pool(name="p_wO", bufs=1)
        p_osb = tc.alloc_tile_pool(name="p_osb", bufs=4)
        p_ave = tc.alloc_tile_pool(name="p_ave", bufs=1, space="PSUM")
        p_scp = tc.alloc_tile_pool(name="p_scp", bufs=2, space="PSUM")
        p_pp = tc.alloc_tile_pool(name="p_pp", bufs=1, space="PSUM")
        p_psD = tc.alloc_tile_pool(name="p_psD", bufs=2, space="PSUM")

        wo_sb = []
        for h in range(HPC):
            w = p_wO.tile([P, OUT_COLS], BF16, name=f"wo_{h}", tag=f"wo_{h}")
            nc.sync.dma_start(out=w[:], in_=wo_d[h * P:(h + 1) * P, :])
            wo_sb.append(w)
        # Lsgn [P, 1] const: +1 at the time row (VSP), -1 elsewhere, so one
        # matmul against sq gives innr = t^2 - sum(space^2) directly.
        Lsgn = p_wO.tile([P, 1], F32, name="L2", tag="L2")
        nc.sync.dma_start(out=Lsgn[:], in_=l2_d[:])

        def sc_pair(g, h, jp, jmax):
            # scores for j-tiles jp, jp+1 into one [P, 2, NCH] fp16 (1-bank)
            # tile. Diagonal tiles compute full 512 q cols (masked cols are
            # real scores, zeroed in ex after the exp).
            c0 = g * NCH
            sc = p_scp.tile([P, 2, NCH], F32, name="sc", tag="sc", bufs=2)
            for dj in range(2):
                j = jp + dj
                nc.tensor.matmul(sc[:, dj, :], ksA[h][:, j * P:(j + 1) * P],
                                 qsA[h][:, c0:c0 + NCH], start=True, stop=False)
                nc.tensor.matmul(sc[:, dj, :], ksB[h][:, j * P:(j + 1) * P],
                                 qsB[h][:, c0:c0 + NCH], start=False, stop=True)
            return sc

        def exp_av(g, h, jp, ave, sc, jmax):
            ex = p_ex.tile([P, 2, NCH], BF16, name="ex", tag="ex", bufs=3)
            nc.scalar.activation(ex[:], sc[:], AF.Exp, scale=exp_scale)
            if causal:
                for dj in range(2):
                    j = jp + dj
                    d = j - g * GQ
                    if d >= 0:
                        if d > 0:
                            nc.vector.memset(ex[:, dj, 0:d * P], 0.0)
                        nc.vector.tensor_mul(ex[:, dj, d * P:(d + 1) * P],
                                             ex[:, dj, d * P:(d + 1) * P],
                                             diagmask[:])
            for dj in range(2):
                j = jp + dj
                nc.tensor.matmul(ave[:], Vp[h][:, j, :], ex[:, dj, :],
                                 start=(j == 0), stop=(j == jmax - 1))

        def epilogue(g, h, ave, cen2):
            # aveT [d, q]: innr per q col = t^2 - sum_d(space^2) > 0 (row VSP
            # is the time coord); one signed-sum matmul against Lsgn gives it
            # directly. rsv = 1/sqrt(innr) is broadcast to 128 partitions by
            # a 1-row outer-product matmul (engines reject stride-0 APs).
            sq = p_cw.tile([P, NCH], F32, name="sq", tag="sq", bufs=2)
            nc.scalar.square(sq[:], ave[:])
            # one PSUM bank reused: innr lands in row 0, then the broadcast
            # matmul overwrites the whole bank after the Ln has consumed it
            ppb = p_pp.tile([P, NCH], F32, name="ppb", tag="ppb", bufs=1)
            nc.tensor.matmul(ppb[0:1, :], Lsgn[:], sq[:], start=True, stop=True)
            lnr = p_cw.tile([1, NCH], F32, name="lnr", tag="lnr", bufs=2)
            nc.scalar.activation(lnr[:], ppb[0:1, :], AF.Ln)
            rsv = p_cw.tile([1, NCH], F32, name="rsv", tag="rsv", bufs=2)
            nc.scalar.activation(rsv[:], lnr[:], AF.Exp, scale=-0.5)
            nc.tensor.matmul(ppb[:], ones_row[:], rsv[:], start=True,
                             stop=True)
            rbs = p_cw.tile([P, NCH], BF16, name="rbs", tag="rbs", bufs=2)
            nc.vector.tensor_copy(rbs[:], ppb[:])
            nc.vector.tensor_mul(cen2[:, h, :], ave[:], rbs[:])

        # wo work is queued as (m, n) jobs and SPRINKLED between the score
        # matmuls and the exp-gated AV matmul: the PE queue then always holds
        # independent work ahead of the semaphore-waiting AV, so the engine
        # never micro-idles (micro-gaps hold the HAM clock gate at half rate).
        wo_jobs = []

        def wo_one(g, cen2, t, n, drain_scalar=False):
            m = g * GQ + t
            n0 = n * NCH
            nn = min(NCH, OUT_COLS - n0)
            ps = p_psD.tile([P, NCH], F32, name="psd", tag="psd", bufs=2)
            nc.tensor.matmul(ps[:, :nn], cen2[:, 0, t * P:(t + 1) * P],
                             wo_sb[0][:, n0:n0 + nn], start=True, stop=False)
            nc.tensor.matmul(ps[:, :nn], cen2[:, 1, t * P:(t + 1) * P],
                             wo_sb[1][:, n0:n0 + nn], start=False, stop=True)
            # drains stay off the scalar engine while the attention loop runs
            # (it gates the exp -> AV chain); the final flush alternates onto
            # the then-idle scalar engine. The 1/256 undoes the two x16
            # fp8 prescales (cen and wo).
            ot = p_osb.tile([P, NCH], BF16, name="ot", tag="ot", bufs=6)
            if drain_scalar:
                nc.scalar.copy(ot[:, :nn], ps[:, :nn])
            else:
                nc.vector.tensor_copy(ot[:, :nn], ps[:, :nn])
            nc.sync.dma_start(out=out_d[m * P:(m + 1) * P, n0:n0 + nn],
                              in_=ot[:, :nn])

        def wo_emit(k=1):
            for _ in range(k):
                if wo_jobs:
                    wo_jobs.pop(0)()

        prev_cen = None
        for g in range(NG):
            cen2 = p_cw.tile([P, 2, NCH], BF16, name="cen2", tag="cen2", bufs=2)
            for h in range(HPC):
                ave = p_ave.tile([P, NCH], F32, name="ave", tag="ave", bufs=1)
                jmax = (g * GQ + GQ) if causal else NQT
                pend = []
                for jp in range(0, jmax, 2):
                    pend.append((jp, sc_pair(g, h, jp, jmax)))
                    wo_emit()
                    if len(pend) > 1:
                        pj, psc = pend.pop(0)
                        exp_av(g, h, pj, ave, psc, jmax)
                for (pj, psc) in pend:
                    wo_emit()
                    exp_av(g, h, pj, ave, psc, jmax)
                epilogue(g, h, ave, cen2)
                wo_emit(2)
                if h == 0 and prev_cen is not None:
                    cp = prev_cen
                    wo_jobs.extend(
                        (lambda t=t, n=n, cp=cp, gg=g - 1: wo_one(gg, cp, t, n))
                        for t in range(GQ) for n in range(4))
            # drain leftovers before the next group's epilogue can wrap the
            # cen2 double-buffer ring
            wo_emit(len(wo_jobs))
            prev_cen = cen2
        for t in range(GQ):
            for n in range(4):
                wo_one(NG - 1, prev_cen, t, n, drain_scalar=(n % 2 == 1))

        p_psD.release()
        p_pp.release()
        p_scp.release()
        p_ave.release()
        p_osb.release()
        p_wO.release()
        p_cw.release()
        p_ex.release()

        big2.release()
        big.release()
        const.release()

    nc.compile()
    return nc


_CACHE = {}


def _get_program(exp_scale: float, causal: bool):
    key = (round(float(exp_scale), 12), causal)
    if key not in _CACHE:
        _CACHE[key] = _build_program(float(exp_scale), causal)
    return _CACHE[key]


def _rope_perm():
    """Even rope dims first, then odd (host-side column permutation)."""
    return np.concatenate([np.arange(0, RSP, 2), np.arange(1, RSP, 2)])


def kernel(x, start_pos, freqs_cos, freqs_sin, mask, wq_w, wkv_a_w, kv_norm_w,
           wkv_b_w, wo_w, softmax_scale, bias_p, _want_trace=False):
    x2 = np.ascontiguousarray(np.asarray(x, np.float32).reshape(S, DIM))
    xT = np.ascontiguousarray(x2.T)
    wq_w = np.asarray(wq_w, np.float32)
    wkv_a_w = np.asarray(wkv_a_w, np.float32)
    kv_norm_w = np.asarray(kv_norm_w, np.float32)
    wkv_b_w = np.asarray(wkv_b_w, np.float32)
    wo_w = np.asarray(wo_w, np.float32)
    cosT = np.asarray(freqs_cos, np.float32).T
    sinT = np.asarray(freqs_sin, np.float32).T
    cosT = np.ascontiguousarray(
        np.concatenate([cosT, cosT], axis=0).astype(np.float16))
    sinT = np.ascontiguousarray(
        np.concatenate([sinT, sinT], axis=0).astype(np.float16))

    mask = np.asarray(mask)
    causal = bool(np.array_equal(mask, np.triu(np.ones((S, S), bool), k=1)))
    if not causal:
        assert not mask.any(), "only causal or empty masks are supported"

    smax = float(np.asarray(softmax_scale).reshape(-1)[0])
    exp_scale = 2.0 / smax

    rp = _rope_perm()
    # wq per core-pair layout: [nope_h0 | nope_h1 | rope_h0(ev,od) | rope_h1(ev,od)]
    wq_r = wq_w.reshape(DIM, 16, QH)
    wq_nope = wq_r[:, :, :NOPE]                       # (DIM, 16, 128)
    wq_rope = wq_r[:, :, NOPE:][:, :, rp]             # (DIM, 16, 64) permuted
    wq_cores = []
    for c in range(N_CORES):
        h0, h1 = 2 * c, 2 * c + 1
        wq_cores.append(np.concatenate(
            [wq_nope[:, h0], wq_nope[:, h1], wq_rope[:, h0], wq_rope[:, h1]],
            axis=1))
    # wkva: [kv | rope-even | rope-odd]
    wkva_p = wkv_a_w.copy()
    wkva_p[:, KV_RANK:] = wkva_p[:, KV_RANK:][:, rp]
    # wkvb: kvn rows first, time row last
    wkvb_p = np.ascontiguousarray(np.concatenate([wkv_b_w[1:], wkv_b_w[:1]], axis=0))
    wnormT = np.ascontiguousarray(kv_norm_w.reshape(4, P).T)
    # wo rows per head: [v space (1..127), time (0)]
    wo_p = wo_w.reshape(16, P, OUT_COLS)
    wo_p = np.concatenate([wo_p[:, 1:, :], wo_p[:, 0:1, :]], axis=1)
    wo_p = wo_p.reshape(16 * P, OUT_COLS)

    nc = _get_program(exp_scale, causal)

    l2c = np.full((P, 1), -1.0, np.float32)
    l2c[VSP, 0] = 1.0

    # one-hot column selectors for the batched [8, 512] time-row reductions:
    # cols 0-63: slab r -> col r ones (full 128 rows), for bsq/qbsq (r=h*4+n)
    # cols 64-95: slab n -> cols {n, 4+n} ones on rows 0-63, for kpsq
    # cols 96-127: slab n -> col n on rows 0-63, col 4+n on rows 64-127 (qsq)
    lcc = np.zeros((P, 128), np.float16)
    for r in range(8):
        lcc[:, 8 * r + r] = 1.0
    for n4 in range(4):
        lcc[0:RSP, 64 + 8 * n4 + n4] = 1.0
        lcc[0:RSP, 64 + 8 * n4 + 4 + n4] = 1.0
        lcc[0:RSP, 96 + 8 * n4 + n4] = 1.0
        lcc[RSP:P, 96 + 8 * n4 + 4 + n4] = 1.0

    xT_bf = np.ascontiguousarray(xT.astype(np.float16))
    xT_f8 = np.ascontiguousarray(xT.astype(ml_dtypes.float8_e4m3))
    wkva_bf = np.ascontiguousarray(wkva_p.astype(np.float16))

    in_maps = []
    for c in range(N_CORES):
        in_maps.append({
            "xT": xT_f8,
            "wq": np.ascontiguousarray(wq_cores[c].astype(np.float16)),
            "wkva": wkva_bf,
            "wnormT": wnormT,
            "wkvb": np.ascontiguousarray(
                wkvb_p[:, c * WB_COLS:(c + 1) * WB_COLS].astype(np.float16)),
            "wo": np.ascontiguousarray(
                wo_p[c * WO_ROWS:(c + 1) * WO_ROWS, :].astype(np.float16)),
            "cosT": cosT,
            "sinT": sinT,
            "l2c": l2c,
            "lcc": lcc,
            "xsl": np.ascontiguousarray(xT_bf[:, c * SL:(c + 1) * SL]),
            "cossl": np.ascontiguousarray(cosT[:, c * SL:(c + 1) * SL]),
            "sinsl": np.ascontiguousarray(sinT[:, c * SL:(c + 1) * SL]),
        })

    res = run_bass_kernel_spmd(nc, in_maps, core_ids=list(range(N_CORES)),
                               trace=_want_trace)
    kernel.last_result = res

    total = res.results[0]["out"].astype(np.float32)
    for c in range(1, N_CORES):
        total = total + res.results[c]["out"].astype(np.float32)
    t = np.sqrt(np.sum(total * total, axis=-1, keepdims=True) + 1.0)
    out = np.concatenate([t, total], axis=-1)
    return out.reshape(1, S, DIM).astype(np.float32)

